# revision 41
# baseline (speedup 1.0000x reference)
"""Trainium2 Bass kernel for nn_LAINRDecoder (sparse attention INR decoder).

The reference's top-128 sparse attention set is a CONTIGUOUS token window
[s, s+128) with s = clip(floor((idx+1)/4) - 64, 0, 896)  (convex quadratic
bias; verified against jax.lax.top_k in test.py).  Sparse gather-attention
therefore equals dense attention with a per-query window mask.

v4 performance structure (on top of the v3 sorted-query sharding):
  * per-HALF token bases: each core's 512 sorted queries are split into two
    256-query halves; each half gets its own 256-token slice (base = that
    half's min window start).  Every query's 128-token window then fits in
    its half's two 128-token chunks, so attention is 2 chunks/query instead
    of 3 (-33% logits/exp/mask/AV volume).  Fallback to the v3 3-chunk
    whole-core program when a half spans > 128 window starts.
  * fp16 weights + tokens + post-softmax activations: halves DMA bytes and
    doubles DVE 16-bit throughput; fp32/fp32r kept where precision matters
    (gamma frequencies, logits before exp, all PSUM accumulation).
  * all input DMAs issued from the scalar (Activation) queue -- the
    earliest-starting HWDGE engine -- in dependency-priority order.
  * HAM clock-gate management: the PE's clock starts at 1.2GHz (K=4/8) and
    only reaches 2.4GHz after ~3.4us of sustained busy; any >0.5us idle gap
    re-throttles it.  Warm-up matmuls (off an iota tile, no input deps)
    start right after the engine preamble, and small always-ready filler
    matmuls bridge known dependency stalls so the PE never idles long
    enough to re-throttle.
  * elementwise work spread across Scalar/DVE/Pool so the scalar engine's
    exp chain (which feeds the PE) isn't queued behind relu/copy traffic.
  * softmax denominators via a ones-augmented AV column; reciprocal as
    exp(-ln(x)) on the scalar engine, Ln reading PSUM directly.
"""

import math
import os
import sys
import types
from contextlib import ExitStack

import numpy as np

# ---------------------------------------------------------------------------
# environment shims (axon NTFF hook + artifact upload are absent in this
# container; inject them so run_bass_kernel_spmd works with trace=True)
# ---------------------------------------------------------------------------
def _install_shims():
    if "antenv.axon_hooks" not in sys.modules:
        hooks = types.ModuleType("antenv.axon_hooks")
        try:
            from trn_agent_boot.trn_boot import _ntff_profile_via_ctypes

            _hook = _ntff_profile_via_ctypes("/opt/axon/libaxon_pjrt.so")
        except Exception:
            _hook = None
        hooks.get_axon_ntff_profile_hook = lambda: _hook
        hooks.set_axon_ntff_profile_hook = lambda h: None
        sys.modules["antenv.axon_hooks"] = hooks
    import concourse.bass_utils as bass_utils

    bass_utils.upload_artifacts = lambda tmpdir: tmpdir


_install_shims()

import concourse.bass as bass
import concourse.mybir as mybir
import concourse.tile as tile
from concourse.bass_utils import run_bass_kernel_spmd

F32 = mybir.dt.float32
F32R = mybir.dt.float32r
F16 = mybir.dt.float16
AF = mybir.ActivationFunctionType
OP = mybir.AluOpType

# problem constants (hardcoded per the harness contract)
B = 2
Q = 4096
L = 1024
HD = 256
FD = 64
INNER = 128
HEADS = 2
DH = 64
TOPK = 128
N_FREQ = 8
LAYER_NUM = 2
N_CORES = 8
QS = Q // N_CORES          # queries per core (512)
HQ = QS // 2               # queries per half (256)
SCALE = DH ** -0.5
NEG_BIG = -1.0e6           # additive mask for out-of-window logits
MAGIC = 1.5 * 2.0 ** 23    # RNE round-to-int magic constant

TWO_PI = 2.0 * math.pi


def _omegas(sigma):
    return np.logspace(1.0, np.log10(sigma), N_FREQ).astype(np.float32)


def _w2(sigma):
    """(5, 64): rows 0-3 arg[grid-dim, out] = pi*omega/2pi (turns); row 4 =
    sin/cos phase (0 or 0.25 turns), contracted against a ones row of qgrid."""
    w = np.zeros((5, 64), np.float32)
    om = _omegas(sigma)
    for c in range(4):
        for j in range(N_FREQ):
            w[c, c * 16 + j] = np.float32(math.pi) * om[j] / np.float32(TWO_PI)
            w[c, c * 16 + 8 + j] = np.float32(math.pi) * om[j] / np.float32(TWO_PI)
        w[4, c * 16 + 8 : c * 16 + 16] = 0.25
    return w


# ---------------------------------------------------------------------------
# v4 weight packs: wf (f32, gamma consts + biases + mask cols),
# wb1 (fp16, attention-path stationaries), wb2 (fp16, tail stationaries)
# ---------------------------------------------------------------------------
def _build_layout4():
    lay = {}
    cols = [0, 0, 0]

    def add(g, name, ncols):
        lay[name] = (g, cols[g], ncols)
        cols[g] += ncols

    # group 0 (f32r): gamma/mask/bias constants
    add(0, "w2q", 64)         # (5,64) incl phase row
    add(0, "w2b", 64)         # (5,64)
    add(0, "ones", 128)       # (1,128) f32 ones row (psA bcast stationary)
    add(0, "mbias", 2)        # (128,2): col c = c*128 - 63.5 (mask Abs bias)
    add(0, "qb", 2)           # (128,2) query_b chunks
    add(0, "outb", 2)         # (128,2) out_b chunks
    add(0, "bandb", 4)        # (128,4) col l*2+i = band_b[l, i*128:(i+1)*128]
    add(0, "modb", 4)         # (128,4) col l*2+i
    add(0, "hvb", 2)          # (128,2)
    # group 1 (fp16): attention-path stationaries
    add(1, "kvW0", 256)       # (128,256) kv_W[0:128,:]
    add(1, "kvW1", 256)       # (128,256) kv_W[128:256,:]
    add(1, "qW0", 128)        # (128,128) q_W[0:128,:] * SCALE
    add(1, "qW1", 128)        # (128,128) q_W[128:256,:] * SCALE
    add(1, "queryW", 256)     # (64,256) query_W
    add(1, "ones16", 64)      # (1,64) fp16 ones (norm bcast stationary)
    # group 2 (fp16): MLP tail stationaries
    add(2, "outWh", 512)      # 2 x (64,256): [h*256] = out_W[h*64:(h+1)*64,:]
    add(2, "modW", 1024)      # 4 x (128,256): [(l*2+k)*256] = mod_W[l, k*128:(k+1)*128, :]
    add(2, "hvW", 512)        # 2 x (128,256): [k*256] = hv_W[0, k*128:(k+1)*128, :]
    add(2, "olW", 4)          # (128,4): col 2*k+l = outl_W[l, k*128:(k+1)*128, 0]
    add(2, "bandW", 512)      # 2 x (64,256): [l*256] = band_W[l]
    add(2, "olb", 1)          # (1,1) sum(outl_b)
    add(2, "onesq", 512)      # (1,512) fp16 ones row (output bias fold)
    return lay, cols


W4_LAYOUT, W4_TOTALS = _build_layout4()


def _pack_weights4(inp):
    W = [np.zeros((128, W4_TOTALS[0]), np.float32),
         np.zeros((128, W4_TOTALS[1]), np.float16),
         np.zeros((128, W4_TOTALS[2]), np.float16)]

    def put(name, arr):
        g, c0, nc_ = W4_LAYOUT[name]
        arr = np.asarray(arr)
        assert arr.shape[-1] <= nc_
        W[g][: arr.shape[0], c0 : c0 + arr.shape[-1]] = arr

    kv_W = np.asarray(inp["kv_W"], np.float32)
    put("kvW0", kv_W[0:128, :])
    put("kvW1", kv_W[128:256, :])
    q_W = np.asarray(inp["q_W"], np.float32) * np.float32(SCALE)
    put("qW0", q_W[0:128, :])
    put("qW1", q_W[128:256, :])
    put("queryW", np.asarray(inp["query_W"], np.float32))
    put("ones16", np.ones((1, 64), np.float16))
    put("w2q", _w2(128.0))
    put("w2b", _w2(32.0))
    put("ones", np.ones((1, 128), np.float32))
    put("mbias", np.broadcast_to(
        np.arange(2, dtype=np.float32) * 128.0 - 63.5, (128, 2)))
    put("qb", np.asarray(inp["query_b"], np.float32).reshape(2, 128).T)
    bb = np.asarray(inp["band_b"], np.float32)
    put("bandb", np.stack([bb[l, i * 128 : (i + 1) * 128]
                           for l in range(2) for i in range(2)], axis=1))
    # out_b folded into mod_b: modulation@mod_W[l] with modulation = o@out_W
    # + out_b contributes the constant out_b@mod_W[l]
    mod_W_f = np.asarray(inp["mod_W"], np.float32)
    out_b = np.asarray(inp["out_b"], np.float32)
    mb = np.asarray(inp["mod_b"], np.float32) + out_b @ mod_W_f
    put("modb", np.stack([mb[l, i * 128 : (i + 1) * 128]
                          for l in range(2) for i in range(2)], axis=1))
    put("hvb", np.asarray(inp["hv_b"], np.float32).reshape(2, 128).T)

    out_W = np.asarray(inp["out_W"], np.float32)
    put("outWh", np.concatenate([out_W[h * 64 : (h + 1) * 64, :]
                                 for h in range(2)], axis=1))
    mod_W = np.asarray(inp["mod_W"], np.float32)
    put("modW", np.concatenate([mod_W[l, k * 128 : (k + 1) * 128, :]
                                for l in range(2) for k in range(2)], axis=1))
    hv_W = np.asarray(inp["hv_W"], np.float32)
    put("hvW", np.concatenate([hv_W[0, k * 128 : (k + 1) * 128, :]
                               for k in range(2)], axis=1))
    outl_W = np.asarray(inp["outl_W"], np.float32)
    ol = np.zeros((128, 4), np.float32)
    for k in range(2):
        for l in range(2):
            ol[:, 2 * k + l] = outl_W[l, k * 128 : (k + 1) * 128, 0]
    put("olW", ol)
    band_W = np.asarray(inp["band_W"], np.float32)
    put("bandW", np.concatenate([band_W[0], band_W[1]], axis=1))
    put("olb", np.asarray([[np.asarray(inp["outl_b"], np.float32).sum()]]))
    put("onesq", np.ones((1, 512), np.float16))
    return W


def _window_starts(x0):
    """s = clip((idx+1)//4 - 64, 0, 896) per query; pure integer index math."""
    g = np.asarray(x0, np.float64)
    z = np.floor(g[:, 0] * 8).astype(np.int64)
    y = np.floor(g[:, 1] * 8).astype(np.int64)
    x = np.floor(g[:, 2] * 8).astype(np.int64)
    t = np.floor(g[:, 3] * 8).astype(np.int64)
    idx = ((t * 8 + z) * 8 + y) * 8 + x
    return np.clip((idx + 1) // 4 - 64, 0, 896)


# ---------------------------------------------------------------------------
# v4 program: two 256-query halves per core, each with its own 256-token
# slice; 2 chunks per half.  fp16 stationaries/activations, f32 logits.
# ---------------------------------------------------------------------------
def build_program4(debug_taps=False):
    LTH = 2 * TOPK            # tokens per half slice (256)
    nc = bass.Bass("TRN2", target_bir_lowering=False, debug=False)
    dbg_d = {}
    if debug_taps:
        for nm, shp in [("d_qT", (128, QS)), ("d_oN", (64, QS)),
                        ("d_hl", (128, QS)), ("d_modT", (128, QS)),
                        ("d_mls", (128, QS)), ("d_hv", (128, QS)),
                        ("d_gq", (64, QS)), ("d_KT", (128, 512)),
                        ("d_lni", (128, QS)), ("d_V", (128, 130)),
                        ("d_D0", (128, QS)), ("d_P0", (128, QS)),
                        ("d_pot", (65, QS))]:
            dbg_d[nm] = nc.dram_tensor(nm, shp, F32, kind="ExternalOutput").ap()

    wf_d = nc.dram_tensor("wf", (128, W4_TOTALS[0]), F32R, kind="ExternalInput").ap()
    wb1_d = nc.dram_tensor("wb1", (128, W4_TOTALS[1]), F16, kind="ExternalInput").ap()
    wb2_d = nc.dram_tensor("wb2", (128, W4_TOTALS[2]), F16, kind="ExternalInput").ap()
    tokp = [nc.dram_tensor(f"tokpack{b}", (128, 4 * LTH), F16,
                           kind="ExternalInput").ap() for b in range(B)]
    qgrid = nc.dram_tensor("qgrid", (5, QS), F32R, kind="ExternalInput").ap()
    qsa = nc.dram_tensor("qsa", (1, QS), F32R, kind="ExternalInput").ap()
    out_d = nc.dram_tensor("out", (B, QS), F32, kind="ExternalOutput").ap()

    ctx = ExitStack()
    with tile.TileContext(nc) as tc:
        cpool = ctx.enter_context(tc.tile_pool(name="consts", bufs=1))
        featp = ctx.enter_context(tc.tile_pool(name="feat", bufs=1))
        kvp = ctx.enter_context(tc.tile_pool(name="kv", bufs=1))
        maskp = ctx.enter_context(tc.tile_pool(name="mask", bufs=1))
        pp = ctx.enter_context(tc.tile_pool(name="pp", bufs=3))
        miscp = ctx.enter_context(tc.tile_pool(name="misc", bufs=2))
        onp = ctx.enter_context(tc.tile_pool(name="on", bufs=1))
        mlp = ctx.enter_context(tc.tile_pool(name="mlt", bufs=2))
        p_big = ctx.enter_context(tc.tile_pool(name="pbig", bufs=3, space="PSUM"))
        p_av = ctx.enter_context(tc.tile_pool(name="pav", bufs=3, space="PSUM"))
        p_row = ctx.enter_context(tc.tile_pool(name="prow", bufs=1, space="PSUM"))
        p_warm = ctx.enter_context(tc.tile_pool(name="pwarm", bufs=1, space="PSUM"))

        # ---- input DMAs: all on the scalar HWDGE queue (earliest user
        # start), in dependency-priority order ----------------------------
        wf = cpool.tile([128, W4_TOTALS[0]], F32R, tag="wf", name="wf")
        nc.scalar.dma_start(wf[:], wf_d[:])
        qg = cpool.tile([5, QS], F32R, tag="qg", name="qg")
        nc.scalar.dma_start(qg[:], qgrid[:])
        qs = cpool.tile([1, QS], F32R, tag="qs", name="qs")
        nc.scalar.dma_start(qs[:], qsa[:])
        wb1 = cpool.tile([128, W4_TOTALS[1]], F16, tag="wb1", name="wb1")
        nc.scalar.dma_start(wb1[:], wb1_d[:])
        tokt = [cpool.tile([128, 4 * LTH], F16, tag=f"tokt{b}", name=f"tokt{b}")
                for b in range(B)]
        for b in range(B):
            nc.scalar.dma_start(tokt[b][:], tokp[b][:])
        wb2 = cpool.tile([128, W4_TOTALS[2]], F16, tag="wb2", name="wb2")
        nc.scalar.dma_start(wb2[:], wb2_d[:])
        wts = [wf, wb1, wb2]

        def wsl(name, p0=0, np_=128, sub=None):
            g, c0, ncols = W4_LAYOUT[name]
            t_ = wts[g]
            if sub is not None:
                c0, ncols = c0 + sub[0], sub[1]
            return t_[p0 : p0 + np_, c0 : c0 + ncols]

        def rf(ap):
            return ap.bitcast(F32)

        # ---- warm-up: iota (no deps) feeds PE immediately so the HAM
        # clock-gate opens during the input-DMA wait.  NOTE: iota must write
        # an F32 tile -- into an f32r tile it stores raw integer bit
        # patterns (denormals), which silently breaks consumers.
        iotaP = cpool.tile([128, QS], F32, tag="iotap", name="iotap")
        nc.gpsimd.iota(iotaP[:], pattern=[[0, QS]], base=0, channel_multiplier=1,
                       allow_small_or_imprecise_dtypes=True)
        for wi in range(7):
            pw = p_warm.tile([128, QS], F32, tag="warm", name=f"warm{wi}")
            nc.tensor.matmul(pw[:], iotaP[:, 0:128], iotaP[:], start=True, stop=True)

        def fill(n, tag):
            """Always-ready PE filler matmuls (~110ns each warm, fp32 N=64)
            to bridge a dependency stall without letting the HAM re-throttle."""
            for i in range(n):
                pw = p_warm.tile([128, 64], F32, tag="warm", name=f"f{tag}{i}")
                nc.tensor.matmul(pw[:], iotaP[:, 0:128], iotaP[:, 0:64],
                                 start=True, stop=True)

        # V tiles: per (batch, half, chunk), [Vh0 | 1 | Vh1 | 1] f32r (the
        # AV moving operand P is f32r, and 32-bit can't mix with fp16); the
        # AV stationary slice for head h is cols [65h, 65h+65) and the
        # denominator lands at pot row 64 for both heads.
        t_V = [[[kvp.tile([128, 130], F32R, tag=f"V{b}{x}{c}", name=f"V{b}{x}{c}")
                 for c in range(2)] for x in range(2)] for b in range(B)]
        for b in range(B):
            for x in range(2):
                for c in range(2):
                    nc.gpsimd.memset(t_V[b][x][c].bitcast(F32)[:, 64:65], 1.0)
                    nc.gpsimd.memset(t_V[b][x][c].bitcast(F32)[:, 129:130], 1.0)

        # ---- query features (dep: qgrid + wf) ---------------------------
        def gamma_T(w2name, tag):
            """(64, QS) fp16 = sin(2pi * frac(turns)); turns from one matmul
            with the phase folded in as a 5th contraction row.  The matmul
            must run in full fp32 (not f32r): turn values reach +-64 and
            f32r's reduced mantissa would quantize the phase by ~0.03."""
            pa = p_big.tile([128, QS], F32, tag="st", name=f"pa_{tag}")
            nc.tensor.matmul(pa[:64, :], rf(wsl(w2name, 0, 5)), rf(qg[:]),
                             start=True, stop=True)
            kf = featp.tile([64, QS], F32, tag=f"{tag}_kf", name=f"{tag}_kf")
            nc.vector.tensor_scalar(kf[:], pa[:64, :], MAGIC, MAGIC,
                                    OP.add, OP.subtract)
            f = featp.tile([64, QS], F32, tag=f"{tag}_f", name=f"{tag}_f")
            nc.vector.tensor_tensor(f[:], pa[:64, :], kf[:], OP.subtract)
            g = featp.tile([64, QS], F16, tag=f"{tag}_g", name=f"{tag}_g")
            nc.scalar.activation(g[:], f[:], AF.Sin, scale=TWO_PI)
            return g

        gq = gamma_T("w2q", "gq")      # used by attention AND band layer 0
        gb1 = gamma_T("w2b", "gb1")    # band layer 1

        # x_qT (256, QS) fp16 = relu(query_W^T @ gammaT + qb)
        x_qT = [featp.tile([128, QS], F16, tag=f"xq{i}", name=f"xq{i}")
                for i in range(2)]
        for i in range(2):
            px = p_big.tile([128, QS], F32, tag="st", name=f"px{i}")
            nc.tensor.matmul(px[:], wsl("queryW", 0, 64, (i * 128, 128)),
                             gq[:], start=True, stop=True)
            nc.scalar.activation(x_qT[i][:], px[:], AF.Relu,
                                 bias=rf(wsl("qb", 0, 128, (i, 1))))
        # qT (128, QS) fp16 = (q_W*scale)^T @ x_qT
        qT = featp.tile([INNER, QS], F16, tag="qT", name="qT")
        pq = p_big.tile([128, QS], F32, tag="st", name="pq")
        for k in range(2):
            nc.tensor.matmul(pq[:], wsl(f"qW{k}"), x_qT[k][:],
                             start=(k == 0), stop=(k == 1))
        nc.vector.tensor_copy(qT[:], pq[:])

        # ---- additive window masks in (token, query) layout -------------
        # D0[p, q] = p - sA[q]; chunk c out-of-window <=> |D0 + 128c - 63.5| > 63.9
        psA = p_big.tile([128, QS], F32, tag="st", name="psA")
        nc.tensor.matmul(psA[:], wsl("ones", 0, 1), qs[:], start=True, stop=True)
        D0 = maskp.tile([128, QS], F32, tag="D0", name="D0")
        nc.vector.tensor_tensor(D0[:], iotaP[:], psA[:], OP.subtract)
        t_lni = []
        for c in range(2):
            ac = miscp.tile([128, QS], F32, tag="ac", name="ac")
            nc.scalar.activation(ac[:], D0[:], AF.Abs,
                                 bias=rf(wsl("mbias", 0, 128, (c, 1))))
            lni = maskp.tile([128, QS], F32, tag=f"lni{c}", name=f"lni{c}")
            nc.gpsimd.tensor_scalar(lni[:], ac[:], 63.9, NEG_BIG, OP.is_gt, OP.mult)
            t_lni.append(lni)

        # ---- KV setup (dep: tokpack[b] + wb1) ---------------------------
        # tokt[b] col layout: [x*2*LTH + k*LTH + t] = tokens[b, cb_x + t, k*128+p]
        t_KT = [kvp.tile([128, 2 * LTH], F16, tag=f"KT{b}", name=f"KT{b}")
                for b in range(B)]

        def emit_kv(b):
            pk = p_big.tile([128, 2 * LTH], F32, tag="st", name=f"pk{b}")
            for x in range(2):
                for k in range(2):
                    nc.tensor.matmul(pk[:, x * LTH : (x + 1) * LTH],
                                     wsl(f"kvW{k}", 0, 128, (0, 128)),
                                     tokt[b][:, (x * 2 + k) * LTH : (x * 2 + k + 1) * LTH],
                                     start=(k == 0), stop=(k == 1))
            nc.vector.tensor_copy(t_KT[b][:], pk[:])
            for x in range(2):
                for c in range(2):
                    pvt = p_big.tile([128, QS], F32, tag="st", name=f"pv{b}{x}{c}")
                    pv = pvt[:, 0:128]
                    for k in range(2):
                        nc.tensor.matmul(
                            pv,
                            tokt[b][:, (x * 2 + k) * LTH + c * 128 :
                                       (x * 2 + k) * LTH + c * 128 + 128],
                            wsl(f"kvW{k}", 0, 128, (128, 128)),
                            start=(k == 0), stop=(k == 1))
                    vdst = t_V[b][x][c][:, 0:130].rearrange(
                        "p (a b) -> p a b", a=2, b=65)[:, :, 0:64]
                    vsrc = pvt[:, 0:128].rearrange("p (a b) -> p a b", a=2, b=64)
                    if (x + c) % 2:
                        nc.vector.tensor_copy(vdst, vsrc)
                    else:
                        nc.scalar.copy(vdst, vsrc)

        emit_kv(0)
        fill(2, "kv")
        emit_kv(1)

        # band features h_lT (2 layers x 2 chunks of (128, QS)) -- emitted
        # here as real PE work that can overlap the attention scalar chains
        h_lT = [[featp.tile([128, QS], F16, tag=f"hl{l}{i}", name=f"hl{l}{i}")
                 for i in range(2)] for l in range(2)]

        def emit_band(l, i):
            gsrc = gq if l == 0 else gb1
            ph = p_big.tile([128, QS], F32, tag="st", name=f"ph{l}{i}")
            nc.tensor.matmul(ph[:], wsl("bandW", 0, 64, (l * 256 + i * 128, 128)),
                             gsrc[:], start=True, stop=True)
            nc.scalar.activation(h_lT[l][i][:], ph[:], AF.Relu,
                                 bias=rf(wsl("bandb", 0, 128, (l * 2 + i, 1))))

        # ---- attention per (batch, head); halves share full-width st ----
        oN = {}      # (b,h) -> fp16 (64, QS) normalized attention out
        pots = {}

        def emit_attn(b, h):
            pot = p_av.tile([65, QS], F32, tag="ot", name=f"ot{b}{h}")
            sts = []
            for c in range(2):
                st = p_big.tile([128, QS], F32, tag="st", name=f"st{b}{h}{c}")
                for x in range(2):
                    nc.tensor.matmul(
                        st[:, x * HQ : (x + 1) * HQ],
                        t_KT[b][h * 64 : (h + 1) * 64,
                                x * LTH + c * 128 : x * LTH + (c + 1) * 128],
                        qT[h * 64 : (h + 1) * 64, x * HQ : (x + 1) * HQ],
                        start=True, stop=True)
                nc.vector.tensor_tensor(st[:], st[:], t_lni[c][:], OP.add)
                # P must be f32r: exp overflows fp16's 65504 for logits > ~11
                # (the reference softmax is max-subtracted; this one is not)
                P = pp.tile([128, QS], F32R, tag="P", name="P")
                nc.scalar.activation(P[:], st[:], AF.Exp)
                sts.append(P)
            # each half's accumulation group must close before the next
            # opens -- interleaved open groups in one PSUM bank lose writes
            for x in range(2):
                for c in range(2):
                    nc.tensor.matmul(
                        pot[:, x * HQ : (x + 1) * HQ],
                        t_V[b][x][c][:, h * 65 : (h + 1) * 65],
                        sts[c][:, x * HQ : (x + 1) * HQ],
                        start=(c == 0), stop=(c == 1))
            if debug_taps and (b, h) == (0, 0):
                tP = mlp.tile([128, QS], F32, tag="tP", name="tP")
                nc.vector.tensor_copy(tP[:], sts[0].bitcast(F32)[:])
                nc.sync.dma_start(dbg_d["d_P0"][:], tP[:])
                tpot = mlp.tile([65, QS], F32, tag="tpot", name="tpot")
                nc.vector.tensor_copy(tpot[:], pot[:])
                nc.sync.dma_start(dbg_d["d_pot"][:], tpot[:])
            # denominator reciprocal via exp(-ln(x)); Ln reads PSUM directly
            dr = miscp.tile([1, QS], F32, tag="dr", name="dr")
            nc.scalar.activation(dr[:], pot[64:65, :], AF.Ln)
            # inv stays f32r: exp(-ln d) underflows fp16 for large denominators
            inv = miscp.tile([1, QS], F32R, tag="inv", name="inv")
            nc.scalar.activation(inv[:], dr[:], AF.Exp, scale=-1.0)
            pots[(b, h)] = (pot, inv)

        def emit_norm(b, h):
            pot, inv = pots[(b, h)]
            pbc = p_big.tile([64, QS], F32, tag="st", name=f"pbc{b}{h}")
            nc.tensor.matmul(pbc[:], wsl("ones", 0, 1, (0, 64)), inv[:],
                             start=True, stop=True)
            bcs = miscp.tile([64, QS], F32, tag="bcs", name="bcs")
            nc.scalar.copy(bcs[:], pbc[:])
            onh = onp.tile([64, QS], F16, tag=f"on{b}{h}", name=f"on{b}{h}")
            nc.vector.tensor_tensor(onh[:], pot[0:64, :], bcs[:], OP.mult)
            oN[(b, h)] = onh[:]

        emit_attn(0, 0)
        emit_band(0, 0)
        emit_attn(0, 1)
        emit_band(0, 1)
        emit_norm(0, 0)
        emit_attn(1, 0)
        emit_band(1, 0)
        emit_norm(0, 1)
        emit_attn(1, 1)
        emit_band(1, 1)

        # ---- MLP tail ----------------------------------------------------
        # modulationT (2 chunks of (128, QS)) = out_W^T @ [oN0; oN1] + out_b
        modT = {}

        def emit_modT(b):
            # out_b is folded into modb host-side, so modT is a plain copy
            for mc in range(2):
                pm = p_big.tile([128, QS], F32, tag="st", name=f"pm{b}{mc}")
                for h in range(2):
                    nc.tensor.matmul(
                        pm[:], wsl("outWh", 0, 64, (h * 256 + mc * 128, 128)),
                        oN[(b, h)], start=(h == 0), stop=(h == 1))
                mt = mlp.tile([128, QS], F16, tag=f"modT{b}{mc}", name=f"modT{b}{mc}")
                nc.vector.tensor_copy(mt[:], pm[:])
                modT[(b, mc)] = mt

        mls = {}

        def emit_mls(b):
            for l in range(2):
                for mc in range(2):
                    pm = p_big.tile([128, QS], F32, tag="st", name=f"pml{b}{l}{mc}")
                    for k in range(2):
                        nc.tensor.matmul(
                            pm[:], wsl("modW", 0, 128,
                                       ((l * 2 + k) * 256 + mc * 128, 128)),
                            modT[(b, k)][:], start=(k == 0), stop=(k == 1))
                    tadd = miscp.tile([128, QS], F16, tag="tadd", name="tadd")
                    nc.vector.scalar_tensor_tensor(
                        tadd[:], pm[:], rf(wsl("modb", 0, 128, (l * 2 + mc, 1))),
                        h_lT[l][mc][:], OP.add, OP.add)
                    ml = mlp.tile([128, QS], F16, tag=f"ml{b}{l}{mc}",
                                  name=f"ml{b}{l}{mc}")
                    nc.gpsimd.tensor_scalar(ml[:], tadd[:], 0.0, None, OP.max)
                    mls[(b, l, mc)] = ml

        sum01 = {}

        def emit_sum01(b):
            for mc in range(2):
                s01 = miscp.tile([128, QS], F16, tag=f"s01{b}{mc}", name=f"s01{b}{mc}")
                nc.gpsimd.tensor_tensor(s01[:], mls[(b, 0, mc)][:],
                                        mls[(b, 1, mc)][:], OP.add)
                sum01[(b, mc)] = s01

        hv1 = {}

        def emit_hv(b):
            for mc in range(2):
                pm = p_big.tile([128, QS], F32, tag="st", name=f"phv{b}{mc}")
                for k in range(2):
                    nc.tensor.matmul(
                        pm[:], wsl("hvW", 0, 128, (k * 256 + mc * 128, 128)),
                        sum01[(b, k)][:], start=(k == 0), stop=(k == 1))
                hv = mlp.tile([128, QS], F16, tag=f"hv{b}{mc}", name=f"hv{b}{mc}")
                nc.scalar.activation(hv[:], pm[:], AF.Relu,
                                     bias=rf(wsl("hvb", 0, 128, (mc, 1))))
                hv1[(b, mc)] = hv

        def emit_out(b):
            # out row = mls0 @ olW0 + hv1 @ olW1 + olb (rank-1 bias fold)
            por = p_row.tile([1, QS], F32, tag="por", name=f"por{b}")
            steps = [(wsl("olW", 0, 128, (2 * k, 1)), mls[(b, 0, k)][:]) for k in range(2)] + \
                    [(wsl("olW", 0, 128, (2 * k + 1, 1)), hv1[(b, k)][:]) for k in range(2)] + \
                    [(wsl("olb", 0, 1), wsl("onesq", 0, 1))]
            for si, (lw, rv) in enumerate(steps):
                nc.tensor.matmul(por[:], lw, rv, start=(si == 0),
                                 stop=(si == len(steps) - 1))
            orow = mlp.tile([1, QS], F32, tag=f"orow{b}", name=f"orow{b}")
            nc.vector.tensor_copy(orow[:], por[:])
            nc.sync.dma_start(out_d[b : b + 1, :], orow[:])

        emit_norm(1, 0)
        emit_modT(0)
        emit_mls(0)
        emit_norm(1, 1)
        emit_sum01(0)
        emit_hv(0)
        emit_modT(1)
        emit_mls(1)
        emit_sum01(1)
        emit_out(0)
        emit_hv(1)
        emit_out(1)
        if debug_taps:
            def tap(name, ap):
                t = mlp.tile(list(ap.shape), F32, tag=f"tp{name}", name=f"tp{name}")
                nc.vector.tensor_copy(t[:], ap.bitcast(F32)
                                      if ap.dtype in (F32R,) else ap)
                nc.sync.dma_start(dbg_d[name][:], t[:])
            tap("d_qT", qT[:])
            tap("d_oN", oN[(0, 0)])
            tap("d_hl", h_lT[0][0][:])
            tap("d_modT", modT[(0, 0)][:])
            tap("d_mls", mls[(0, 0, 0)][:])
            tap("d_hv", hv1[(0, 0)][:])
            tap("d_gq", gq[:])
            tap("d_KT", t_KT[0][:, 0:512])
            tap("d_lni", t_lni[0][:])
            tap("d_V", t_V[0][0][0].bitcast(F32)[:])
            tap("d_D0", D0[:])
        ctx.close()

    _split_multi_waits_inline(nc)
    return nc


# ---------------------------------------------------------------------------
# v3 fallback program (3 chunks, whole-core token slice, f32r) -- used when
# an input distribution gives a half more than 128 distinct window starts.
# ---------------------------------------------------------------------------
def _build_layout3():
    lay = {}
    cols = [0, 0, 0]

    def add(g, name, ncols):
        lay[name] = (g, cols[g], ncols)
        cols[g] += ncols

    add(0, "w2q", 64)
    add(0, "w2b", 64)
    add(0, "scb", 1)
    add(0, "ones", 128)
    add(0, "mbias", 8)
    add(1, "kvW0", 256)
    add(1, "kvW1", 256)
    add(1, "qW0", 128)
    add(1, "qW1", 128)
    add(1, "queryW", 256)
    add(1, "qb", 2)
    add(1, "onescol", 1)
    add(1, "onesq", 512)
    add(2, "modW", 1024)
    add(2, "hvW", 512)
    add(2, "olW", 4)
    add(2, "outWh", 512)
    add(2, "bandW", 512)
    add(2, "outb", 2)
    add(2, "bandb", 4)
    add(2, "modb", 4)
    add(2, "hvb", 2)
    add(2, "olb", 1)
    return lay, cols


W_LAYOUT, W_TOTALS = _build_layout3()


def _w2_v3(sigma):
    w = np.zeros((4, 64), np.float32)
    om = _omegas(sigma)
    for c in range(4):
        for j in range(N_FREQ):
            w[c, c * 16 + j] = np.float32(math.pi) * om[j]
            w[c, c * 16 + 8 + j] = np.float32(math.pi) * om[j]
    return w


def _sincos_bias():
    b = np.zeros((64, 1), np.float32)
    for c in range(4):
        b[c * 16 + 8 : c * 16 + 16, 0] = np.float32(math.pi / 2)
    return b


def _pack_weights3(inp):
    W = [np.zeros((128, W_TOTALS[g]), np.float32) for g in range(3)]

    def put(name, arr):
        g, c0, nc_ = W_LAYOUT[name]
        arr = np.asarray(arr, np.float32)
        assert arr.shape[-1] <= nc_
        W[g][: arr.shape[0], c0 : c0 + arr.shape[-1]] = arr

    kv_W = np.asarray(inp["kv_W"], np.float32)
    put("kvW0", kv_W[0:128, :])
    put("kvW1", kv_W[128:256, :])
    q_W = np.asarray(inp["q_W"], np.float32)
    put("qW0", q_W[0:128, :])
    put("qW1", q_W[128:256, :])
    put("queryW", np.asarray(inp["query_W"], np.float32))
    put("w2q", _w2_v3(128.0) / TWO_PI)
    put("w2b", _w2_v3(32.0) / TWO_PI)
    put("qb", np.asarray(inp["query_b"], np.float32).reshape(2, 128).T)
    put("scb", _sincos_bias() / TWO_PI)
    put("ones", np.ones((1, 128), np.float32))
    put("mbias", np.broadcast_to(
        np.arange(8, dtype=np.float32) * 128.0 - 63.5, (128, 8)))
    put("onescol", np.ones((128, 1), np.float32))
    put("onesq", np.ones((1, 512), np.float32))

    mod_W = np.asarray(inp["mod_W"], np.float32)
    put("modW", np.concatenate([mod_W[l, k * 128 : (k + 1) * 128, :]
                                for l in range(2) for k in range(2)], axis=1))
    hv_W = np.asarray(inp["hv_W"], np.float32)
    put("hvW", np.concatenate([hv_W[0, k * 128 : (k + 1) * 128, :]
                               for k in range(2)], axis=1))
    outl_W = np.asarray(inp["outl_W"], np.float32)
    ol = np.zeros((128, 4), np.float32)
    for k in range(2):
        for l in range(2):
            ol[:, 2 * k + l] = outl_W[l, k * 128 : (k + 1) * 128, 0]
    put("olW", ol)
    out_W = np.asarray(inp["out_W"], np.float32)
    put("outWh", np.concatenate([out_W[h * 64 : (h + 1) * 64, :]
                                 for h in range(2)], axis=1))
    band_W = np.asarray(inp["band_W"], np.float32)
    put("bandW", np.concatenate([band_W[0], band_W[1]], axis=1))
    put("outb", np.asarray(inp["out_b"], np.float32).reshape(2, 128).T)
    bb = np.asarray(inp["band_b"], np.float32)
    put("bandb", np.stack([bb[l, i * 128 : (i + 1) * 128]
                           for l in range(2) for i in range(2)], axis=1))
    mb = np.asarray(inp["mod_b"], np.float32)
    put("modb", np.stack([mb[l, i * 128 : (i + 1) * 128]
                          for l in range(2) for i in range(2)], axis=1))
    put("hvb", np.asarray(inp["hv_b"], np.float32).reshape(2, 128).T)
    put("olb", np.asarray([[np.asarray(inp["outl_b"], np.float32).sum()]]))
    return W


def build_program3(nch):
    """v3 fallback: nch 128-token chunks for all 512 queries of a core."""
    LT = nch * 128
    nc = bass.Bass("TRN2", target_bir_lowering=False, debug=False)

    wp = [nc.dram_tensor(f"wpack{g}", (128, W_TOTALS[g]), F32R,
                         kind="ExternalInput").ap() for g in range(3)]
    tokp = [nc.dram_tensor(f"tokpack{b}", (128, 2 * LT), F32R,
                           kind="ExternalInput").ap() for b in range(B)]
    qgrid = nc.dram_tensor("qgrid", (4, QS), F32R, kind="ExternalInput").ap()
    qsa = nc.dram_tensor("qsa", (1, QS), F32R, kind="ExternalInput").ap()
    out_d = nc.dram_tensor("out", (B, QS), F32, kind="ExternalOutput").ap()

    ctx = ExitStack()
    with tile.TileContext(nc) as tc:
        cpool = ctx.enter_context(tc.tile_pool(name="consts", bufs=1))
        featp = ctx.enter_context(tc.tile_pool(name="feat", bufs=1))
        kvp = ctx.enter_context(tc.tile_pool(name="kv", bufs=1))
        maskp = ctx.enter_context(tc.tile_pool(name="mask", bufs=1))
        ep = ctx.enter_context(tc.tile_pool(name="ep", bufs=3))
        miscp = ctx.enter_context(tc.tile_pool(name="misc", bufs=2))
        pp = ctx.enter_context(tc.tile_pool(name="pp", bufs=4))
        onp = ctx.enter_context(tc.tile_pool(name="on", bufs=1))
        mlp = ctx.enter_context(tc.tile_pool(name="mlt", bufs=2))
        p_big = ctx.enter_context(tc.tile_pool(name="pbig", bufs=3, space="PSUM"))
        p_av = ctx.enter_context(tc.tile_pool(name="pav", bufs=3, space="PSUM"))
        p_row = ctx.enter_context(tc.tile_pool(name="prow", bufs=2, space="PSUM"))

        qg = cpool.tile([4, QS], F32R, tag="qg", name="qg")
        nc.sync.dma_start(qg[:], qgrid[:])
        wt0 = cpool.tile([128, W_TOTALS[0]], F32R, tag="wt0", name="wt0")
        nc.sync.dma_start(wt0[:], wp[0][:])
        qs = cpool.tile([1, QS], F32R, tag="qs", name="qs")
        nc.sync.dma_start(qs[:], qsa[:])
        tokt = [cpool.tile([128, 2 * LT], F32R, tag=f"tokt{b}", name=f"tokt{b}")
                for b in range(B)]
        for b in range(B):
            nc.scalar.dma_start(tokt[b][:], tokp[b][:])
        wt1 = cpool.tile([128, W_TOTALS[1]], F32R, tag="wt1", name="wt1")
        nc.scalar.dma_start(wt1[:], wp[1][:])
        wt2 = cpool.tile([128, W_TOTALS[2]], F32R, tag="wt2", name="wt2")
        nc.scalar.dma_start(wt2[:], wp[2][:])
        wts = [wt0, wt1, wt2]

        def wsl(name, p0=0, np_=128, sub=None):
            g, c0, ncols = W_LAYOUT[name]
            t_ = wts[g]
            if sub is not None:
                c0, ncols = c0 + sub[0], sub[1]
            return t_[p0 : p0 + np_, c0 : c0 + ncols]

        def rf(ap):
            return ap.bitcast(F32)

        warm = cpool.tile([128, QS], F32, tag="warm", name="warm")
        nc.gpsimd.memset(warm[:], 0.0)
        for wi in range(6):
            pw = p_big.tile([128, QS], F32, tag="st", name=f"warm{wi}")
            nc.tensor.matmul(pw[:], warm[:, 0:128], warm[:], start=True, stop=True)
        iotaP = cpool.tile([128, QS], F32, tag="iotap", name="iotap")
        nc.gpsimd.iota(iotaP[:], pattern=[[0, QS]], base=0, channel_multiplier=1,
                       allow_small_or_imprecise_dtypes=True)

        t_V = [[kvp.tile([128, 130], F32R, tag=f"V{b}{c}", name=f"V{b}{c}")
                for c in range(nch)] for b in range(B)]
        for b in range(B):
            for c in range(nch):
                nc.scalar.copy(t_V[b][c][:, 64:65], rf(wsl("onescol")))
                nc.scalar.copy(t_V[b][c][:, 129:130], rf(wsl("onescol")))

        def gamma_T(w2name, tag):
            pa = p_big.tile([128, QS], F32, tag="st", name=f"pa_{tag}")
            nc.tensor.matmul(pa[:64, :], rf(wsl(w2name, 0, 4)), rf(qg[:]),
                             start=True, stop=True)
            u0 = featp.tile([64, QS], F32, tag=f"{tag}_u0", name=f"{tag}_u0")
            nc.vector.tensor_scalar(u0[:], pa[:64, :], rf(wsl("scb", 0, 64)),
                                    None, OP.add)
            kf = featp.tile([64, QS], F32, tag=f"{tag}_kf", name=f"{tag}_kf")
            nc.vector.tensor_scalar(kf[:], u0[:], MAGIC, MAGIC, OP.add, OP.subtract)
            f = featp.tile([64, QS], F32, tag=f"{tag}_f", name=f"{tag}_f")
            nc.vector.tensor_tensor(f[:], u0[:], kf[:], OP.subtract)
            g = featp.tile([64, QS], F32R, tag=f"{tag}_g", name=f"{tag}_g")
            nc.scalar.activation(g[:], f[:], AF.Sin, scale=TWO_PI)
            return g

        gq = gamma_T("w2q", "gq")
        gb1 = gamma_T("w2b", "gb1")

        t_KT = [kvp.tile([128, LT], F32R, tag=f"KT{b}", name=f"KT{b}")
                for b in range(B)]
        for b in range(B):
            pk = p_big.tile([128, LT], F32, tag="st", name=f"pk{b}")
            for k in range(2):
                nc.tensor.matmul(pk[:], wsl(f"kvW{k}", 0, 128, (0, 128)),
                                 tokt[b][:, k * LT : (k + 1) * LT],
                                 start=(k == 0), stop=(k == 1))
            nc.scalar.copy(t_KT[b][:], pk[:])
            for c in range(nch):
                pvt = p_big.tile([128, QS], F32, tag="st", name=f"pv{b}{c}")
                pv = pvt[:, 0:128]
                for k in range(2):
                    nc.tensor.matmul(
                        pv, tokt[b][:, k * LT + c * 128 : k * LT + c * 128 + 128],
                        wsl(f"kvW{k}", 0, 128, (128, 128)),
                        start=(k == 0), stop=(k == 1))
                nc.vector.tensor_copy(t_V[b][c][:, 0:64], pvt[:, 0:64])
                nc.vector.tensor_copy(t_V[b][c][:, 65:129], pvt[:, 64:128])

        h_lT = [[featp.tile([128, QS], F32, tag=f"hl{l}{i}", name=f"hl{l}{i}")
                 for i in range(2)] for l in range(2)]
        x_qT = [featp.tile([128, QS], F32R, tag=f"xq{i}", name=f"xq{i}")
                for i in range(2)]
        for i in range(2):
            px = p_big.tile([128, QS], F32, tag="st", name=f"px{i}")
            nc.tensor.matmul(px[:], wsl("queryW", 0, 64, (i * 128, 128)),
                             gq[:], start=True, stop=True)
            nc.scalar.activation(x_qT[i][:], px[:], AF.Relu,
                                 bias=rf(wsl("qb", 0, 128, (i, 1))))
        for i in range(2):
            ph = p_big.tile([128, QS], F32, tag="st", name=f"ph0{i}")
            nc.tensor.matmul(ph[:], wsl("bandW", 0, 64, (0 * 256 + i * 128, 128)),
                             gq[:], start=True, stop=True)
            nc.scalar.activation(h_lT[0][i][:], ph[:], AF.Relu,
                                 bias=rf(wsl("bandb", 0, 128, (0 * 2 + i, 1))))
        qT = featp.tile([INNER, QS], F32R, tag="qT", name="qT")
        pq = p_big.tile([128, QS], F32, tag="st", name="pq")
        for k in range(2):
            nc.tensor.matmul(pq[:], wsl(f"qW{k}"), x_qT[k][:],
                             start=(k == 0), stop=(k == 1))
        nc.scalar.activation(qT[:], pq[:], AF.Copy, scale=SCALE)
        for i in range(2):
            ph = p_big.tile([128, QS], F32, tag="st", name=f"ph1{i}")
            nc.tensor.matmul(ph[:], wsl("bandW", 0, 64, (1 * 256 + i * 128, 128)),
                             gb1[:], start=True, stop=True)
            nc.scalar.activation(h_lT[1][i][:], ph[:], AF.Relu,
                                 bias=rf(wsl("bandb", 0, 128, (1 * 2 + i, 1))))
        psA = p_big.tile([128, QS], F32, tag="st", name="psA")
        nc.tensor.matmul(psA[:], wsl("ones", 0, 1), qs[:], start=True, stop=True)
        D0 = maskp.tile([128, QS], F32, tag="D0", name="D0")
        nc.vector.tensor_tensor(D0[:], iotaP[:], psA[:], OP.subtract)
        t_lni = []
        for c in range(nch):
            ac = miscp.tile([128, QS], F32, tag="ac", name="ac")
            nc.scalar.activation(ac[:], D0[:], AF.Abs,
                                 bias=rf(wsl("mbias", 0, 128, (c, 1))))
            lni = maskp.tile([128, QS], F32, tag=f"lni{c}", name=f"lni{c}")
            nc.vector.tensor_scalar(lni[:], ac[:], 63.9, NEG_BIG, OP.is_gt, OP.mult)
            t_lni.append(lni)

        oN = {}

        def emit_norm(b, h, pot, inv):
            pbc = p_big.tile([64, QS], F32, tag="st", name=f"pbc{b}{h}")
            nc.tensor.matmul(pbc[:], wsl("ones", 0, 1, (0, 64)), inv[:],
                             start=True, stop=True)
            bcs = miscp.tile([64, QS], F32, tag="bcs", name="bcs")
            nc.scalar.copy(bcs[:], pbc[:])
            onh = onp.tile([64, QS], F32R, tag=f"on{b}{h}", name=f"on{b}{h}")
            nc.vector.tensor_tensor(onh[:], pot[0:64, :], bcs[:], OP.mult)
            oN[(b, h)] = onh

        pend = None
        for b in range(B):
            for h in range(HEADS):
                pot = p_av.tile([65, QS], F32, tag="ot", name=f"ot{b}{h}")
                for c in range(nch):
                    st = p_big.tile([128, QS], F32, tag="st", name=f"st{b}{h}{c}")
                    nc.tensor.matmul(
                        st[:],
                        t_KT[b][h * 64 : (h + 1) * 64, c * 128 : (c + 1) * 128],
                        qT[h * 64 : (h + 1) * 64, :],
                        start=True, stop=True)
                    sm = ep.tile([128, QS], F32, tag="sm", name="sm")
                    nc.any.tensor_tensor(sm[:], st[:], t_lni[c][:], OP.add)
                    P = pp.tile([128, QS], F32R, tag="P", name="P")
                    nc.scalar.activation(P[:], sm[:], AF.Exp)
                    nc.tensor.matmul(
                        pot[:], t_V[b][c][:, h * 65 : (h + 1) * 65], P[:],
                        start=(c == 0), stop=(c == nch - 1))
                dr = miscp.tile([1, QS], F32, tag="dr", name="dr")
                nc.any.tensor_copy(dr[:], pot[64:65, :])
                nc.scalar.activation(dr[:], dr[:], AF.Ln)
                inv = miscp.tile([1, QS], F32R, tag="inv", name="inv")
                nc.scalar.activation(inv[:], dr[:], AF.Exp, scale=-1.0)
                if pend is not None:
                    emit_norm(*pend)
                pend = (b, h, pot, inv)

        modT = {}
        for b in range(B):
            for mc in range(2):
                pm = p_big.tile([128, QS], F32, tag="st", name=f"pm{b}{mc}")
                for h in range(2):
                    nc.tensor.matmul(
                        pm[:], wsl("outWh", 0, 64, (h * 256 + mc * 128, 128)),
                        oN[(b, h)][:], start=(h == 0), stop=(h == 1))
                mt = mlp.tile([128, QS], F32R, tag=f"modT{b}{mc}", name=f"modT{b}{mc}")
                nc.scalar.activation(mt[:], pm[:], AF.Identity,
                                     bias=rf(wsl("outb", 0, 128, (mc, 1))))
                modT[(b, mc)] = mt
            if b == 0:
                emit_norm(*pend)
        mlsd = {}
        for b in range(B):
            for l in range(2):
                for mc in range(2):
                    pm = p_big.tile([128, QS], F32, tag="st", name=f"pml{b}{l}{mc}")
                    for k in range(2):
                        nc.tensor.matmul(
                            pm[:], wsl("modW", 0, 128,
                                       ((l * 2 + k) * 256 + mc * 128, 128)),
                            modT[(b, k)][:], start=(k == 0), stop=(k == 1))
                    tadd = miscp.tile([128, QS], F32, tag="tadd", name="tadd")
                    nc.any.tensor_tensor(tadd[:], pm[:], h_lT[l][mc][:], OP.add)
                    ml = mlp.tile([128, QS], F32R, tag=f"ml{b}{l}{mc}",
                                  name=f"ml{b}{l}{mc}")
                    nc.scalar.activation(ml[:], tadd[:], AF.Relu,
                                         bias=rf(wsl("modb", 0, 128, (l * 2 + mc, 1))))
                    mlsd[(b, l, mc)] = ml
        sum01 = {}
        for b in range(B):
            for mc in range(2):
                s01 = miscp.tile([128, QS], F32R, tag=f"s01{b}{mc}", name=f"s01{b}{mc}")
                nc.any.tensor_tensor(s01[:], rf(mlsd[(b, 0, mc)][:]),
                                     rf(mlsd[(b, 1, mc)][:]), OP.add)
                sum01[(b, mc)] = s01
        hv1 = {}
        for b in range(B):
            for mc in range(2):
                pm = p_big.tile([128, QS], F32, tag="st", name=f"phv{b}{mc}")
                for k in range(2):
                    nc.tensor.matmul(
                        pm[:], wsl("hvW", 0, 128, (k * 256 + mc * 128, 128)),
                        sum01[(b, k)][:], start=(k == 0), stop=(k == 1))
                hv = mlp.tile([128, QS], F32R, tag=f"hv{b}{mc}", name=f"hv{b}{mc}")
                nc.scalar.activation(hv[:], pm[:], AF.Relu,
                                     bias=rf(wsl("hvb", 0, 128, (mc, 1))))
                hv1[(b, mc)] = hv
        for b in range(B):
            por = p_row.tile([1, QS], F32, tag="por", name=f"por{b}")
            steps = [(wsl("olW", 0, 128, (2 * k, 1)), mlsd[(b, 0, k)][:]) for k in range(2)] + \
                    [(wsl("olW", 0, 128, (2 * k + 1, 1)), hv1[(b, k)][:]) for k in range(2)] + \
                    [(wsl("olb", 0, 1), wsl("onesq", 0, 1))]
            for si, (lw, rv) in enumerate(steps):
                nc.tensor.matmul(por[:], lw, rv, start=(si == 0),
                                 stop=(si == len(steps) - 1))
            orow = mlp.tile([1, QS], F32, tag=f"orow{b}", name=f"orow{b}")
            nc.vector.tensor_copy(orow[:], por[:])
            nc.sync.dma_start(out_d[b : b + 1, :], orow[:])
        ctx.close()

    _split_multi_waits_inline(nc)
    return nc


def _split_multi_waits_inline(nc):
    """Split multi-semaphore waits into NOP chains (one wait each)."""
    for fn in nc.m.functions:
        for blk in fn.blocks:
            new_insts = []
            for inst in blk.instructions:
                si = getattr(inst, "sync_info", None)
                if si is not None and len(si.on_wait) > 1:
                    waits = list(si.on_wait)
                    for j, w in enumerate(waits[:-1]):
                        new_insts.append(mybir.InstNoOp(
                            name=f"{inst.name}-ws{j}",
                            engine=inst.engine,
                            sync_info=mybir.SyncInfo(on_wait=[w], on_update=[]),
                            bass_nofuse=True,
                        ))
                    si.on_wait = waits[-1:]
                new_insts.append(inst)
            blk.instructions = new_insts


_CACHED = {}
LAST_RESULTS = None


def _run_v4(inputs, x0, s_all, order):
    """Half-based program; returns None if the input needs the fallback."""
    cbases = []          # (core, half) -> token base
    for c in range(N_CORES):
        for x in range(2):
            qidx = order[c * QS + x * HQ : c * QS + (x + 1) * HQ]
            smin, smax = int(s_all[qidx].min()), int(s_all[qidx].max())
            cb = max(0, min(smin, L - 2 * TOPK))
            if smax - cb > TOPK:          # window must fit 2 chunks
                return None
            cbases.append(cb)

    if "v4" not in _CACHED:
        _CACHED["v4"] = build_program4()
    nc = _CACHED["v4"]

    wpacks = _pack_weights4(inputs)
    tokens = np.asarray(inputs["tokens"], np.float32)
    tt = np.ascontiguousarray(tokens.transpose(0, 2, 1)).astype(np.float16)
    LTH = 2 * TOPK

    in_maps = []
    for c in range(N_CORES):
        qidx = order[c * QS : (c + 1) * QS]
        cbA, cbB = cbases[2 * c], cbases[2 * c + 1]
        srel = np.concatenate([
            (s_all[qidx[:HQ]] - cbA), (s_all[qidx[HQ:]] - cbB)]).astype(np.float32)
        g5 = np.concatenate([x0[qidx].T, np.ones((1, QS), np.float32)], axis=0)
        m = {"wf": wpacks[0], "wb1": wpacks[1], "wb2": wpacks[2],
             "qgrid": np.ascontiguousarray(g5),
             "qsa": np.ascontiguousarray(srel[None, :])}
        for b in range(B):
            m[f"tokpack{b}"] = np.ascontiguousarray(np.concatenate(
                [tt[b, 0:128, cbA : cbA + LTH],
                 tt[b, 128:256, cbA : cbA + LTH],
                 tt[b, 0:128, cbB : cbB + LTH],
                 tt[b, 128:256, cbB : cbB + LTH]], axis=1))
        in_maps.append(m)

    trace = bool(os.environ.get("KERNEL_TRACE"))
    return run_bass_kernel_spmd(nc, in_maps, core_ids=list(range(N_CORES)),
                                trace=trace)


def _run_v3(inputs, x0, s_all, order):
    cbases, needs = [], []
    for c in range(N_CORES):
        qidx = order[c * QS : (c + 1) * QS]
        smin, smax = int(s_all[qidx].min()), int(s_all[qidx].max())
        needs.append(smax + TOPK - smin)
        cbases.append(smin)
    nch = max(3, int(math.ceil(max(needs) / 128.0)))
    LT = nch * 128
    cbases = [max(0, min(cb, L - LT)) for cb in cbases]

    key = ("v3", nch)
    if key not in _CACHED:
        _CACHED[key] = build_program3(nch)
    nc = _CACHED[key]

    wpacks = _pack_weights3(inputs)
    tokens = np.asarray(inputs["tokens"], np.float32)
    tt = tokens.transpose(0, 2, 1)

    in_maps = []
    for c in range(N_CORES):
        qidx = order[c * QS : (c + 1) * QS]
        cb = cbases[c]
        m = {"wpack0": wpacks[0], "wpack1": wpacks[1], "wpack2": wpacks[2],
             "qgrid": np.ascontiguousarray(x0[qidx].T),
             "qsa": np.ascontiguousarray(
                 (s_all[qidx] - cb).astype(np.float32)[None, :])}
        for b in range(B):
            m[f"tokpack{b}"] = np.ascontiguousarray(
                np.concatenate([tt[b, 0:128, cb : cb + LT],
                                tt[b, 128:256, cb : cb + LT]], axis=1))
        in_maps.append(m)

    trace = bool(os.environ.get("KERNEL_TRACE"))
    return run_bass_kernel_spmd(nc, in_maps, core_ids=list(range(N_CORES)),
                                trace=trace)


def kernel(**inputs):
    x = np.asarray(inputs["x"], np.float32)
    assert int(inputs["gD"]) == 8 and int(inputs["gH"]) == 8
    assert int(inputs["gW"]) == 8 and int(inputs["gT"]) == 8

    x0 = np.ascontiguousarray(x[0])  # (Q, 4) -- reference uses x[0] for all batches
    s_all = _window_starts(x0)
    order = np.argsort(s_all, kind="stable")

    global LAST_RESULTS
    res = _run_v4(inputs, x0, s_all, order)
    if res is None:
        res = _run_v3(inputs, x0, s_all, order)
    LAST_RESULTS = res
    out = np.empty((B, Q), np.float32)
    for c in range(N_CORES):
        out[:, order[c * QS : (c + 1) * QS]] = res.results[c]["out"]
    return out.reshape(B, Q, 1)


# revision 47
# speedup vs baseline: 1.9774x; 1.9774x over previous
"""Trainium2 Bass kernel for nn_LAINRDecoder (sparse attention INR decoder).

The reference's top-128 sparse attention set is a CONTIGUOUS token window
[s, s+128) with s = clip(floor((idx+1)/4) - 64, 0, 896)  (convex quadratic
bias; verified against jax.lax.top_k in test.py).  Sparse gather-attention
therefore equals dense attention with a per-query window mask.

v4 performance structure (on top of the v3 sorted-query sharding):
  * per-HALF token bases: each core's 512 sorted queries are split into two
    256-query halves; each half gets its own 256-token slice (base = that
    half's min window start).  Every query's 128-token window then fits in
    its half's two 128-token chunks, so attention is 2 chunks/query instead
    of 3 (-33% logits/exp/mask/AV volume).  Fallback to the v3 3-chunk
    whole-core program when a half spans > 128 window starts.
  * fp16 weights + tokens + post-softmax activations: halves DMA bytes and
    doubles DVE 16-bit throughput; fp32/fp32r kept where precision matters
    (gamma frequencies, logits before exp, all PSUM accumulation).
  * all input DMAs issued from the scalar (Activation) queue -- the
    earliest-starting HWDGE engine -- in dependency-priority order.
  * HAM clock-gate management: the PE's clock starts at 1.2GHz (K=4/8) and
    only reaches 2.4GHz after ~3.4us of sustained busy; any >0.5us idle gap
    re-throttles it.  Warm-up matmuls (off an iota tile, no input deps)
    start right after the engine preamble, and small always-ready filler
    matmuls bridge known dependency stalls so the PE never idles long
    enough to re-throttle.
  * elementwise work spread across Scalar/DVE/Pool so the scalar engine's
    exp chain (which feeds the PE) isn't queued behind relu/copy traffic.
  * softmax denominators via a ones-augmented AV column; reciprocal as
    exp(-ln(x)) on the scalar engine, Ln reading PSUM directly.
"""

import math
import os
import sys
import types
from contextlib import ExitStack

import numpy as np

# ---------------------------------------------------------------------------
# environment shims (axon NTFF hook + artifact upload are absent in this
# container; inject them so run_bass_kernel_spmd works with trace=True)
# ---------------------------------------------------------------------------
def _install_shims():
    if "antenv.axon_hooks" not in sys.modules:
        hooks = types.ModuleType("antenv.axon_hooks")
        try:
            from trn_agent_boot.trn_boot import _ntff_profile_via_ctypes

            _hook = _ntff_profile_via_ctypes("/opt/axon/libaxon_pjrt.so")
        except Exception:
            _hook = None
        hooks.get_axon_ntff_profile_hook = lambda: _hook
        hooks.set_axon_ntff_profile_hook = lambda h: None
        sys.modules["antenv.axon_hooks"] = hooks
    import concourse.bass_utils as bass_utils

    bass_utils.upload_artifacts = lambda tmpdir: tmpdir


_install_shims()

import concourse.bass as bass
import concourse.mybir as mybir
import concourse.tile as tile
from concourse.bass_utils import run_bass_kernel_spmd

F32 = mybir.dt.float32
F32R = mybir.dt.float32r
F16 = mybir.dt.float16
AF = mybir.ActivationFunctionType
OP = mybir.AluOpType

# problem constants (hardcoded per the harness contract)
B = 2
Q = 4096
L = 1024
HD = 256
FD = 64
INNER = 128
HEADS = 2
DH = 64
TOPK = 128
N_FREQ = 8
LAYER_NUM = 2
N_CORES = 8
QS = Q // N_CORES          # queries per core (512)
HQ = QS // 2               # queries per half (256)
SCALE = DH ** -0.5
NEG_BIG = -1.0e6           # additive mask for out-of-window logits
MAGIC = 1.5 * 2.0 ** 23    # RNE round-to-int magic constant

TWO_PI = 2.0 * math.pi


def _omegas(sigma):
    return np.logspace(1.0, np.log10(sigma), N_FREQ).astype(np.float32)


def _w2(sigma):
    """(5, 64): rows 0-3 arg[grid-dim, out] = pi*omega/2pi (turns); row 4 =
    sin/cos phase (0 or 0.25 turns), contracted against a ones row of qgrid."""
    w = np.zeros((5, 64), np.float32)
    om = _omegas(sigma)
    for c in range(4):
        for j in range(N_FREQ):
            w[c, c * 16 + j] = np.float32(math.pi) * om[j] / np.float32(TWO_PI)
            w[c, c * 16 + 8 + j] = np.float32(math.pi) * om[j] / np.float32(TWO_PI)
        w[4, c * 16 + 8 : c * 16 + 16] = 0.25
    return w


# ---------------------------------------------------------------------------
# v4 weight packs: wf (f32, gamma consts + biases + mask cols),
# wb1 (fp16, attention-path stationaries), wb2 (fp16, tail stationaries)
# ---------------------------------------------------------------------------
def _build_layout4():
    lay = {}
    cols = [0, 0, 0]

    def add(g, name, ncols):
        lay[name] = (g, cols[g], ncols)
        cols[g] += ncols

    # group 0 (f32r): gamma/mask/bias constants
    add(0, "w2q", 64)         # (5,64) incl phase row
    add(0, "w2b", 64)         # (5,64)
    add(0, "ones", 128)       # (1,128) f32 ones row (psA bcast stationary)
    add(0, "mbias", 2)        # (128,2): col c = c*128 - 63.5 (mask Abs bias)
    add(0, "qb", 2)           # (128,2) query_b chunks
    add(0, "outb", 2)         # (128,2) out_b chunks
    add(0, "bandb", 4)        # (128,4) col l*2+i = band_b[l, i*128:(i+1)*128]
    add(0, "modb", 4)         # (128,4) col l*2+i
    add(0, "hvb", 2)          # (128,2)
    # group 1 (fp16): attention-path stationaries
    add(1, "kvW0", 256)       # (128,256) kv_W[0:128,:]
    add(1, "kvW1", 256)       # (128,256) kv_W[128:256,:]
    add(1, "qW0", 128)        # (128,128) q_W[0:128,:] * SCALE
    add(1, "qW1", 128)        # (128,128) q_W[128:256,:] * SCALE
    add(1, "queryW", 256)     # (64,256) query_W
    add(1, "ones16", 64)      # (1,64) fp16 ones (norm bcast stationary)
    # group 2 (fp16): MLP tail stationaries
    add(2, "outWh", 512)      # 2 x (64,256): [h*256] = out_W[h*64:(h+1)*64,:]
    add(2, "modW", 1024)      # 4 x (128,256): [(l*2+k)*256] = mod_W[l, k*128:(k+1)*128, :]
    add(2, "hvW", 512)        # 2 x (128,256): [k*256] = hv_W[0, k*128:(k+1)*128, :]
    add(2, "olW", 4)          # (128,4): col 2*k+l = outl_W[l, k*128:(k+1)*128, 0]
    add(2, "bandW", 512)      # 2 x (64,256): [l*256] = band_W[l]
    add(2, "olb", 1)          # (1,1) sum(outl_b)
    add(2, "onesq", 512)      # (1,512) fp16 ones row (output bias fold)
    add(2, "ident", 128)      # (128,128) fp16 identity (h_l fold via PE)
    return lay, cols


W4_LAYOUT, W4_TOTALS = _build_layout4()


def _pack_weights4(inp):
    W = [np.zeros((128, W4_TOTALS[0]), np.float32),
         np.zeros((128, W4_TOTALS[1]), np.float16),
         np.zeros((128, W4_TOTALS[2]), np.float16)]

    def put(name, arr):
        g, c0, nc_ = W4_LAYOUT[name]
        arr = np.asarray(arr)
        assert arr.shape[-1] <= nc_
        W[g][: arr.shape[0], c0 : c0 + arr.shape[-1]] = arr

    kv_W = np.asarray(inp["kv_W"], np.float32)
    put("kvW0", kv_W[0:128, :])
    put("kvW1", kv_W[128:256, :])
    q_W = np.asarray(inp["q_W"], np.float32) * np.float32(SCALE)
    put("qW0", q_W[0:128, :])
    put("qW1", q_W[128:256, :])
    put("queryW", np.asarray(inp["query_W"], np.float32))
    put("ones16", np.ones((1, 64), np.float16))
    put("w2q", _w2(128.0))
    put("w2b", _w2(32.0))
    put("ones", np.ones((1, 128), np.float32))
    put("mbias", np.broadcast_to(
        np.arange(2, dtype=np.float32) * 128.0 - 63.5, (128, 2)))
    put("qb", np.asarray(inp["query_b"], np.float32).reshape(2, 128).T)
    bb = np.asarray(inp["band_b"], np.float32)
    put("bandb", np.stack([bb[l, i * 128 : (i + 1) * 128]
                           for l in range(2) for i in range(2)], axis=1))
    # out_b folded into mod_b: modulation@mod_W[l] with modulation = o@out_W
    # + out_b contributes the constant out_b@mod_W[l]
    mod_W_f = np.asarray(inp["mod_W"], np.float32)
    out_b = np.asarray(inp["out_b"], np.float32)
    mb = np.asarray(inp["mod_b"], np.float32) + out_b @ mod_W_f
    put("modb", np.stack([mb[l, i * 128 : (i + 1) * 128]
                          for l in range(2) for i in range(2)], axis=1))
    put("hvb", np.asarray(inp["hv_b"], np.float32).reshape(2, 128).T)

    out_W = np.asarray(inp["out_W"], np.float32)
    put("outWh", np.concatenate([out_W[h * 64 : (h + 1) * 64, :]
                                 for h in range(2)], axis=1))
    mod_W = np.asarray(inp["mod_W"], np.float32)
    put("modW", np.concatenate([mod_W[l, k * 128 : (k + 1) * 128, :]
                                for l in range(2) for k in range(2)], axis=1))
    hv_W = np.asarray(inp["hv_W"], np.float32)
    put("hvW", np.concatenate([hv_W[0, k * 128 : (k + 1) * 128, :]
                               for k in range(2)], axis=1))
    outl_W = np.asarray(inp["outl_W"], np.float32)
    ol = np.zeros((128, 4), np.float32)
    for k in range(2):
        for l in range(2):
            ol[:, 2 * k + l] = outl_W[l, k * 128 : (k + 1) * 128, 0]
    put("olW", ol)
    band_W = np.asarray(inp["band_W"], np.float32)
    put("bandW", np.concatenate([band_W[0], band_W[1]], axis=1))
    put("olb", np.asarray([[np.asarray(inp["outl_b"], np.float32).sum()]]))
    put("onesq", np.ones((1, 512), np.float16))
    put("ident", np.eye(128, dtype=np.float16))
    return W


def _window_starts(x0):
    """s = clip((idx+1)//4 - 64, 0, 896) per query; pure integer index math."""
    g = np.asarray(x0, np.float64)
    z = np.floor(g[:, 0] * 8).astype(np.int64)
    y = np.floor(g[:, 1] * 8).astype(np.int64)
    x = np.floor(g[:, 2] * 8).astype(np.int64)
    t = np.floor(g[:, 3] * 8).astype(np.int64)
    idx = ((t * 8 + z) * 8 + y) * 8 + x
    return np.clip((idx + 1) // 4 - 64, 0, 896)


# ---------------------------------------------------------------------------
# v4 program: two 256-query halves per core, each with its own 256-token
# slice; 2 chunks per half.  fp16 stationaries/activations, f32 logits.
# ---------------------------------------------------------------------------
def build_program4(debug_taps=False):
    LTH = 2 * TOPK            # tokens per half slice (256)
    nc = bass.Bass("TRN2", target_bir_lowering=False, debug=False)
    dbg_d = {}
    if debug_taps:
        for nm, shp in [("d_qT", (128, QS)), ("d_oN", (64, QS)),
                        ("d_hl", (128, QS)), ("d_modT", (128, QS)),
                        ("d_mls", (128, QS)), ("d_hv", (128, QS)),
                        ("d_gq", (64, QS)), ("d_KT", (128, 512)),
                        ("d_lni", (128, QS)), ("d_V", (128, 130)),
                        ("d_D0", (128, QS)), ("d_P0", (128, QS)),
                        ("d_pot", (65, QS))]:
            dbg_d[nm] = nc.dram_tensor(nm, shp, F32, kind="ExternalOutput").ap()

    wf_d = nc.dram_tensor("wf", (128, W4_TOTALS[0]), F32R, kind="ExternalInput").ap()
    wb1_d = nc.dram_tensor("wb1", (128, W4_TOTALS[1]), F16, kind="ExternalInput").ap()
    wb2_d = nc.dram_tensor("wb2", (128, W4_TOTALS[2]), F16, kind="ExternalInput").ap()
    tokp = [nc.dram_tensor(f"tokpack{b}", (128, 4 * LTH), F16,
                           kind="ExternalInput").ap() for b in range(B)]
    qgrid = nc.dram_tensor("qgrid", (5, QS), F32R, kind="ExternalInput").ap()
    qsa = nc.dram_tensor("qsa", (1, QS), F32R, kind="ExternalInput").ap()
    out_d = nc.dram_tensor("out", (B, QS), F32, kind="ExternalOutput").ap()

    ctx = ExitStack()
    with tile.TileContext(nc) as tc:
        cpool = ctx.enter_context(tc.tile_pool(name="consts", bufs=1))
        featp = ctx.enter_context(tc.tile_pool(name="feat", bufs=1))
        kvp = ctx.enter_context(tc.tile_pool(name="kv", bufs=1))
        maskp = ctx.enter_context(tc.tile_pool(name="mask", bufs=1))
        pp = ctx.enter_context(tc.tile_pool(name="pp", bufs=3))
        miscp = ctx.enter_context(tc.tile_pool(name="misc", bufs=2))
        onp = ctx.enter_context(tc.tile_pool(name="on", bufs=1))
        mlp = ctx.enter_context(tc.tile_pool(name="mlt", bufs=2))
        p_big = ctx.enter_context(tc.tile_pool(name="pbig", bufs=3, space="PSUM"))
        p_av = ctx.enter_context(tc.tile_pool(name="pav", bufs=3, space="PSUM"))
        p_row = ctx.enter_context(tc.tile_pool(name="prow", bufs=1, space="PSUM"))
        p_warm = ctx.enter_context(tc.tile_pool(name="pwarm", bufs=1, space="PSUM"))

        # ---- input DMAs: all on the scalar HWDGE queue (earliest user
        # start), in dependency-priority order ----------------------------
        wf = cpool.tile([128, W4_TOTALS[0]], F32R, tag="wf", name="wf")
        nc.scalar.dma_start(wf[:], wf_d[:])
        qg = cpool.tile([5, QS], F32R, tag="qg", name="qg")
        nc.scalar.dma_start(qg[:], qgrid[:])
        qs = cpool.tile([1, QS], F32R, tag="qs", name="qs")
        nc.scalar.dma_start(qs[:], qsa[:])
        wb1 = cpool.tile([128, W4_TOTALS[1]], F16, tag="wb1", name="wb1")
        nc.scalar.dma_start(wb1[:], wb1_d[:])
        tokt = [cpool.tile([128, 4 * LTH], F16, tag=f"tokt{b}", name=f"tokt{b}")
                for b in range(B)]
        for b in range(B):
            nc.scalar.dma_start(tokt[b][:], tokp[b][:])
        wb2 = cpool.tile([128, W4_TOTALS[2]], F16, tag="wb2", name="wb2")
        nc.scalar.dma_start(wb2[:], wb2_d[:])
        wts = [wf, wb1, wb2]

        def wsl(name, p0=0, np_=128, sub=None):
            g, c0, ncols = W4_LAYOUT[name]
            t_ = wts[g]
            if sub is not None:
                c0, ncols = c0 + sub[0], sub[1]
            return t_[p0 : p0 + np_, c0 : c0 + ncols]

        def rf(ap):
            return ap.bitcast(F32)

        # ---- warm-up: iota (no deps) feeds PE immediately so the HAM
        # clock-gate opens during the input-DMA wait.  NOTE: iota must write
        # an F32 tile -- into an f32r tile it stores raw integer bit
        # patterns (denormals), which silently breaks consumers.
        iotaP = cpool.tile([128, QS], F32, tag="iotap", name="iotap")
        nc.gpsimd.iota(iotaP[:], pattern=[[0, QS]], base=0, channel_multiplier=1,
                       allow_small_or_imprecise_dtypes=True)
        for wi in range(7):
            pw = p_warm.tile([128, QS], F32, tag="warm", name=f"warm{wi}")
            nc.tensor.matmul(pw[:], iotaP[:, 0:128], iotaP[:], start=True, stop=True)

        def fill(n, tag):
            """Always-ready PE filler matmuls (~110ns each warm, fp32 N=64)
            to bridge a dependency stall without letting the HAM re-throttle."""
            for i in range(n):
                pw = p_warm.tile([128, 64], F32, tag="warm", name=f"f{tag}{i}")
                nc.tensor.matmul(pw[:], iotaP[:, 0:128], iotaP[:, 0:64],
                                 start=True, stop=True)

        # V tiles: per (batch, half, chunk), [Vh0 | 1 | Vh1 | 1] f32r (the
        # AV moving operand P is f32r, and 32-bit can't mix with fp16); the
        # AV stationary slice for head h is cols [65h, 65h+65) and the
        # denominator lands at pot row 64 for both heads.
        t_V = [[[kvp.tile([128, 130], F32R, tag=f"V{b}{x}{c}", name=f"V{b}{x}{c}")
                 for c in range(2)] for x in range(2)] for b in range(B)]
        for b in range(B):
            for x in range(2):
                for c in range(2):
                    nc.gpsimd.memset(t_V[b][x][c].bitcast(F32)[:, 64:65], 1.0)
                    nc.gpsimd.memset(t_V[b][x][c].bitcast(F32)[:, 129:130], 1.0)

        # ---- query features (dep: qgrid + wf) ---------------------------
        def gamma_T(w2name, tag):
            """(64, QS) fp16 = sin(2pi * frac(turns)); turns from one matmul
            with the phase folded in as a 5th contraction row.  The matmul
            must run in full fp32 (not f32r): turn values reach +-64 and
            f32r's reduced mantissa would quantize the phase by ~0.03."""
            pa = p_big.tile([128, QS], F32, tag="st", name=f"pa_{tag}")
            nc.tensor.matmul(pa[:64, :], rf(wsl(w2name, 0, 5)), rf(qg[:]),
                             start=True, stop=True)
            kf = featp.tile([64, QS], F32, tag=f"{tag}_kf", name=f"{tag}_kf")
            nc.vector.tensor_scalar(kf[:], pa[:64, :], MAGIC, MAGIC,
                                    OP.add, OP.subtract)
            f = featp.tile([64, QS], F32, tag=f"{tag}_f", name=f"{tag}_f")
            nc.vector.tensor_tensor(f[:], pa[:64, :], kf[:], OP.subtract)
            g = featp.tile([64, QS], F16, tag=f"{tag}_g", name=f"{tag}_g")
            nc.scalar.activation(g[:], f[:], AF.Sin, scale=TWO_PI)
            return g

        gq = gamma_T("w2q", "gq")      # used by attention AND band layer 0
        gb1 = gamma_T("w2b", "gb1")    # band layer 1

        # x_qT (256, QS) fp16 = relu(query_W^T @ gammaT + qb)
        x_qT = [featp.tile([128, QS], F16, tag=f"xq{i}", name=f"xq{i}")
                for i in range(2)]
        for i in range(2):
            px = p_big.tile([128, QS], F32, tag="st", name=f"px{i}")
            nc.tensor.matmul(px[:], wsl("queryW", 0, 64, (i * 128, 128)),
                             gq[:], start=True, stop=True)
            nc.scalar.activation(x_qT[i][:], px[:], AF.Relu,
                                 bias=rf(wsl("qb", 0, 128, (i, 1))))
        # qT (128, QS) fp16 = (q_W*scale)^T @ x_qT
        qT = featp.tile([INNER, QS], F16, tag="qT", name="qT")
        pq = p_big.tile([128, QS], F32, tag="st", name="pq")
        for k in range(2):
            nc.tensor.matmul(pq[:], wsl(f"qW{k}"), x_qT[k][:],
                             start=(k == 0), stop=(k == 1))
        nc.vector.tensor_copy(qT[:], pq[:])

        # ---- additive window masks in (token, query) layout -------------
        # D0[p, q] = p - sA[q]; chunk c out-of-window <=> |D0 + 128c - 63.5| > 63.9
        psA = p_big.tile([128, QS], F32, tag="st", name="psA")
        nc.tensor.matmul(psA[:], wsl("ones", 0, 1), qs[:], start=True, stop=True)
        D0 = maskp.tile([128, QS], F32, tag="D0", name="D0")
        nc.vector.tensor_tensor(D0[:], iotaP[:], psA[:], OP.subtract)
        t_lni = []
        for c in range(2):
            ac = miscp.tile([128, QS], F32, tag="ac", name="ac")
            nc.scalar.activation(ac[:], D0[:], AF.Abs,
                                 bias=rf(wsl("mbias", 0, 128, (c, 1))))
            lni = maskp.tile([128, QS], F32, tag=f"lni{c}", name=f"lni{c}")
            nc.vector.tensor_scalar(lni[:], ac[:], 63.9, NEG_BIG, OP.is_gt, OP.mult)
            t_lni.append(lni)

        # ---- KV setup (dep: tokpack[b] + wb1) ---------------------------
        # tokt[b] col layout: [x*2*LTH + k*LTH + t] = tokens[b, cb_x + t, k*128+p]
        t_KT = [kvp.tile([128, 2 * LTH], F16, tag=f"KT{b}", name=f"KT{b}")
                for b in range(B)]

        def emit_kv(b):
            pk = p_big.tile([128, 2 * LTH], F32, tag="st", name=f"pk{b}")
            for x in range(2):
                for k in range(2):
                    nc.tensor.matmul(pk[:, x * LTH : (x + 1) * LTH],
                                     wsl(f"kvW{k}", 0, 128, (0, 128)),
                                     tokt[b][:, (x * 2 + k) * LTH : (x * 2 + k + 1) * LTH],
                                     start=(k == 0), stop=(k == 1))
            nc.vector.tensor_copy(t_KT[b][:], pk[:])
            for x in range(2):
                for c in range(2):
                    pvt = p_big.tile([128, QS], F32, tag="st", name=f"pv{b}{x}{c}")
                    pv = pvt[:, 0:128]
                    for k in range(2):
                        nc.tensor.matmul(
                            pv,
                            tokt[b][:, (x * 2 + k) * LTH + c * 128 :
                                       (x * 2 + k) * LTH + c * 128 + 128],
                            wsl(f"kvW{k}", 0, 128, (128, 128)),
                            start=(k == 0), stop=(k == 1))
                    vdst = t_V[b][x][c][:, 0:130].rearrange(
                        "p (a b) -> p a b", a=2, b=65)[:, :, 0:64]
                    vsrc = pvt[:, 0:128].rearrange("p (a b) -> p a b", a=2, b=64)
                    if (x + c) % 2:
                        nc.vector.tensor_copy(vdst, vsrc)
                    else:
                        nc.scalar.copy(vdst, vsrc)

        emit_kv(0)
        fill(2, "kv")
        emit_kv(1)

        # band features h_lT (2 layers x 2 chunks of (128, QS)) -- emitted
        # here as real PE work that can overlap the attention scalar chains
        h_lT = [[featp.tile([128, QS], F16, tag=f"hl{l}{i}", name=f"hl{l}{i}")
                 for i in range(2)] for l in range(2)]

        def emit_band(l, i):
            gsrc = gq if l == 0 else gb1
            ph = p_big.tile([128, QS], F32, tag="st", name=f"ph{l}{i}")
            nc.tensor.matmul(ph[:], wsl("bandW", 0, 64, (l * 256 + i * 128, 128)),
                             gsrc[:], start=True, stop=True)
            nc.scalar.activation(h_lT[l][i][:], ph[:], AF.Relu,
                                 bias=rf(wsl("bandb", 0, 128, (l * 2 + i, 1))))

        # ---- attention per (batch, head); halves share full-width st ----
        oN = {}      # (b,h) -> fp16 (64, QS) normalized attention out
        pots = {}

        def emit_attn(b, h):
            pot = p_av.tile([65, QS], F32, tag="ot", name=f"ot{b}{h}")
            sts = []
            for c in range(2):
                st = p_big.tile([128, QS], F32, tag="st", name=f"st{b}{h}{c}")
                for x in range(2):
                    nc.tensor.matmul(
                        st[:, x * HQ : (x + 1) * HQ],
                        t_KT[b][h * 64 : (h + 1) * 64,
                                x * LTH + c * 128 : x * LTH + (c + 1) * 128],
                        qT[h * 64 : (h + 1) * 64, x * HQ : (x + 1) * HQ],
                        start=True, stop=True)
                nc.vector.tensor_tensor(st[:], st[:], t_lni[c][:], OP.add)
                # P must be f32r: exp overflows fp16's 65504 for logits > ~11
                # (the reference softmax is max-subtracted; this one is not)
                P = pp.tile([128, QS], F32R, tag="P", name="P")
                nc.scalar.activation(P[:], st[:], AF.Exp)
                sts.append(P)
            # each half's accumulation group must close before the next
            # opens -- interleaved open groups in one PSUM bank lose writes
            for x in range(2):
                for c in range(2):
                    nc.tensor.matmul(
                        pot[:, x * HQ : (x + 1) * HQ],
                        t_V[b][x][c][:, h * 65 : (h + 1) * 65],
                        sts[c][:, x * HQ : (x + 1) * HQ],
                        start=(c == 0), stop=(c == 1))
            if debug_taps and (b, h) == (0, 0):
                tP = mlp.tile([128, QS], F32, tag="tP", name="tP")
                nc.vector.tensor_copy(tP[:], sts[0].bitcast(F32)[:])
                nc.sync.dma_start(dbg_d["d_P0"][:], tP[:])
                tpot = mlp.tile([65, QS], F32, tag="tpot", name="tpot")
                nc.vector.tensor_copy(tpot[:], pot[:])
                nc.sync.dma_start(dbg_d["d_pot"][:], tpot[:])
            # denominator reciprocal via exp(-ln(x)); Ln reads PSUM directly
            dr = miscp.tile([1, QS], F32, tag="dr", name="dr")
            nc.scalar.activation(dr[:], pot[64:65, :], AF.Ln)
            # inv stays f32r: exp(-ln d) underflows fp16 for large denominators
            inv = miscp.tile([1, QS], F32R, tag="inv", name="inv")
            nc.scalar.activation(inv[:], dr[:], AF.Exp, scale=-1.0)
            pots[(b, h)] = (pot, inv)

        def emit_norm(b, h):
            pot, inv = pots[(b, h)]
            pbc = p_big.tile([64, QS], F32, tag="st", name=f"pbc{b}{h}")
            nc.tensor.matmul(pbc[:], wsl("ones", 0, 1, (0, 64)), inv[:],
                             start=True, stop=True)
            bcs = miscp.tile([64, QS], F32, tag="bcs", name="bcs")
            nc.scalar.copy(bcs[:], pbc[:])
            onh = onp.tile([64, QS], F16, tag=f"on{b}{h}", name=f"on{b}{h}")
            nc.vector.tensor_tensor(onh[:], pot[0:64, :], bcs[:], OP.mult)
            oN[(b, h)] = onh[:]

        emit_attn(0, 0)
        emit_band(0, 0)
        emit_attn(0, 1)
        emit_band(0, 1)
        emit_norm(0, 0)
        emit_attn(1, 0)
        emit_band(1, 0)
        emit_norm(0, 1)
        emit_attn(1, 1)
        emit_band(1, 1)

        # ---- MLP tail ----------------------------------------------------
        # modulationT (2 chunks of (128, QS)) = out_W^T @ [oN0; oN1] + out_b
        modT = {}

        def emit_modT(b):
            # out_b is folded into modb host-side, so modT is a plain copy
            for mc in range(2):
                pm = p_big.tile([128, QS], F32, tag="st", name=f"pm{b}{mc}")
                for h in range(2):
                    nc.tensor.matmul(
                        pm[:], wsl("outWh", 0, 64, (h * 256 + mc * 128, 128)),
                        oN[(b, h)], start=(h == 0), stop=(h == 1))
                mt = mlp.tile([128, QS], F16, tag=f"modT{b}{mc}", name=f"modT{b}{mc}")
                nc.vector.tensor_copy(mt[:], pm[:])
                modT[(b, mc)] = mt

        mls = {}

        def emit_mls(b):
            # h_l is folded into the PE accumulation via an identity matmul,
            # so the activation is a single bias+relu from PSUM
            for l in range(2):
                for mc in range(2):
                    pm = p_big.tile([128, QS], F32, tag="st", name=f"pml{b}{l}{mc}")
                    for k in range(2):
                        nc.tensor.matmul(
                            pm[:], wsl("modW", 0, 128,
                                       ((l * 2 + k) * 256 + mc * 128, 128)),
                            modT[(b, k)][:], start=(k == 0), stop=False)
                    nc.tensor.matmul(pm[:], wsl("ident"), h_lT[l][mc][:],
                                     start=False, stop=True)
                    ml = mlp.tile([128, QS], F16, tag=f"ml{b}{l}{mc}",
                                  name=f"ml{b}{l}{mc}")
                    nc.scalar.activation(ml[:], pm[:], AF.Relu,
                                         bias=rf(wsl("modb", 0, 128, (l * 2 + mc, 1))))
                    mls[(b, l, mc)] = ml

        hv1 = {}

        def emit_hv(b):
            # hv_W @ (mls0 + mls1) distributed over the sum: 4 accumulation
            # matmuls instead of a separate DVE add
            for mc in range(2):
                pm = p_big.tile([128, QS], F32, tag="st", name=f"phv{b}{mc}")
                for si, (l, k) in enumerate([(l, k) for l in range(2)
                                             for k in range(2)]):
                    nc.tensor.matmul(
                        pm[:], wsl("hvW", 0, 128, (k * 256 + mc * 128, 128)),
                        mls[(b, l, k)][:], start=(si == 0), stop=(si == 3))
                hv = mlp.tile([128, QS], F16, tag=f"hv{b}{mc}", name=f"hv{b}{mc}")
                nc.scalar.activation(hv[:], pm[:], AF.Relu,
                                     bias=rf(wsl("hvb", 0, 128, (mc, 1))))
                hv1[(b, mc)] = hv

        def emit_out(b):
            # out row = mls0 @ olW0 + hv1 @ olW1 + olb (rank-1 bias fold)
            por = p_row.tile([1, QS], F32, tag="por", name=f"por{b}")
            steps = [(wsl("olW", 0, 128, (2 * k, 1)), mls[(b, 0, k)][:]) for k in range(2)] + \
                    [(wsl("olW", 0, 128, (2 * k + 1, 1)), hv1[(b, k)][:]) for k in range(2)] + \
                    [(wsl("olb", 0, 1), wsl("onesq", 0, 1))]
            for si, (lw, rv) in enumerate(steps):
                nc.tensor.matmul(por[:], lw, rv, start=(si == 0),
                                 stop=(si == len(steps) - 1))
            orow = mlp.tile([1, QS], F32, tag=f"orow{b}", name=f"orow{b}")
            nc.vector.tensor_copy(orow[:], por[:])
            nc.sync.dma_start(out_d[b : b + 1, :], orow[:])

        emit_norm(1, 0)
        emit_modT(0)
        emit_mls(0)
        emit_norm(1, 1)
        emit_hv(0)
        emit_modT(1)
        emit_mls(1)
        emit_out(0)
        emit_hv(1)
        emit_out(1)
        if debug_taps:
            def tap(name, ap):
                t = mlp.tile(list(ap.shape), F32, tag=f"tp{name}", name=f"tp{name}")
                nc.vector.tensor_copy(t[:], ap.bitcast(F32)
                                      if ap.dtype in (F32R,) else ap)
                nc.sync.dma_start(dbg_d[name][:], t[:])
            tap("d_qT", qT[:])
            tap("d_oN", oN[(0, 0)])
            tap("d_hl", h_lT[0][0][:])
            tap("d_modT", modT[(0, 0)][:])
            tap("d_mls", mls[(0, 0, 0)][:])
            tap("d_hv", hv1[(0, 0)][:])
            tap("d_gq", gq[:])
            tap("d_KT", t_KT[0][:, 0:512])
            tap("d_lni", t_lni[0][:])
            tap("d_V", t_V[0][0][0].bitcast(F32)[:])
            tap("d_D0", D0[:])
        ctx.close()

    _split_multi_waits_inline(nc)
    return nc


# ---------------------------------------------------------------------------
# v3 fallback program (3 chunks, whole-core token slice, f32r) -- used when
# an input distribution gives a half more than 128 distinct window starts.
# ---------------------------------------------------------------------------
def _build_layout3():
    lay = {}
    cols = [0, 0, 0]

    def add(g, name, ncols):
        lay[name] = (g, cols[g], ncols)
        cols[g] += ncols

    add(0, "w2q", 64)
    add(0, "w2b", 64)
    add(0, "scb", 1)
    add(0, "ones", 128)
    add(0, "mbias", 8)
    add(1, "kvW0", 256)
    add(1, "kvW1", 256)
    add(1, "qW0", 128)
    add(1, "qW1", 128)
    add(1, "queryW", 256)
    add(1, "qb", 2)
    add(1, "onescol", 1)
    add(1, "onesq", 512)
    add(2, "modW", 1024)
    add(2, "hvW", 512)
    add(2, "olW", 4)
    add(2, "outWh", 512)
    add(2, "bandW", 512)
    add(2, "outb", 2)
    add(2, "bandb", 4)
    add(2, "modb", 4)
    add(2, "hvb", 2)
    add(2, "olb", 1)
    return lay, cols


W_LAYOUT, W_TOTALS = _build_layout3()


def _w2_v3(sigma):
    w = np.zeros((4, 64), np.float32)
    om = _omegas(sigma)
    for c in range(4):
        for j in range(N_FREQ):
            w[c, c * 16 + j] = np.float32(math.pi) * om[j]
            w[c, c * 16 + 8 + j] = np.float32(math.pi) * om[j]
    return w


def _sincos_bias():
    b = np.zeros((64, 1), np.float32)
    for c in range(4):
        b[c * 16 + 8 : c * 16 + 16, 0] = np.float32(math.pi / 2)
    return b


def _pack_weights3(inp):
    W = [np.zeros((128, W_TOTALS[g]), np.float32) for g in range(3)]

    def put(name, arr):
        g, c0, nc_ = W_LAYOUT[name]
        arr = np.asarray(arr, np.float32)
        assert arr.shape[-1] <= nc_
        W[g][: arr.shape[0], c0 : c0 + arr.shape[-1]] = arr

    kv_W = np.asarray(inp["kv_W"], np.float32)
    put("kvW0", kv_W[0:128, :])
    put("kvW1", kv_W[128:256, :])
    q_W = np.asarray(inp["q_W"], np.float32)
    put("qW0", q_W[0:128, :])
    put("qW1", q_W[128:256, :])
    put("queryW", np.asarray(inp["query_W"], np.float32))
    put("w2q", _w2_v3(128.0) / TWO_PI)
    put("w2b", _w2_v3(32.0) / TWO_PI)
    put("qb", np.asarray(inp["query_b"], np.float32).reshape(2, 128).T)
    put("scb", _sincos_bias() / TWO_PI)
    put("ones", np.ones((1, 128), np.float32))
    put("mbias", np.broadcast_to(
        np.arange(8, dtype=np.float32) * 128.0 - 63.5, (128, 8)))
    put("onescol", np.ones((128, 1), np.float32))
    put("onesq", np.ones((1, 512), np.float32))

    mod_W = np.asarray(inp["mod_W"], np.float32)
    put("modW", np.concatenate([mod_W[l, k * 128 : (k + 1) * 128, :]
                                for l in range(2) for k in range(2)], axis=1))
    hv_W = np.asarray(inp["hv_W"], np.float32)
    put("hvW", np.concatenate([hv_W[0, k * 128 : (k + 1) * 128, :]
                               for k in range(2)], axis=1))
    outl_W = np.asarray(inp["outl_W"], np.float32)
    ol = np.zeros((128, 4), np.float32)
    for k in range(2):
        for l in range(2):
            ol[:, 2 * k + l] = outl_W[l, k * 128 : (k + 1) * 128, 0]
    put("olW", ol)
    out_W = np.asarray(inp["out_W"], np.float32)
    put("outWh", np.concatenate([out_W[h * 64 : (h + 1) * 64, :]
                                 for h in range(2)], axis=1))
    band_W = np.asarray(inp["band_W"], np.float32)
    put("bandW", np.concatenate([band_W[0], band_W[1]], axis=1))
    put("outb", np.asarray(inp["out_b"], np.float32).reshape(2, 128).T)
    bb = np.asarray(inp["band_b"], np.float32)
    put("bandb", np.stack([bb[l, i * 128 : (i + 1) * 128]
                           for l in range(2) for i in range(2)], axis=1))
    mb = np.asarray(inp["mod_b"], np.float32)
    put("modb", np.stack([mb[l, i * 128 : (i + 1) * 128]
                          for l in range(2) for i in range(2)], axis=1))
    put("hvb", np.asarray(inp["hv_b"], np.float32).reshape(2, 128).T)
    put("olb", np.asarray([[np.asarray(inp["outl_b"], np.float32).sum()]]))
    return W


def build_program3(nch):
    """v3 fallback: nch 128-token chunks for all 512 queries of a core."""
    LT = nch * 128
    nc = bass.Bass("TRN2", target_bir_lowering=False, debug=False)

    wp = [nc.dram_tensor(f"wpack{g}", (128, W_TOTALS[g]), F32R,
                         kind="ExternalInput").ap() for g in range(3)]
    tokp = [nc.dram_tensor(f"tokpack{b}", (128, 2 * LT), F32R,
                           kind="ExternalInput").ap() for b in range(B)]
    qgrid = nc.dram_tensor("qgrid", (4, QS), F32R, kind="ExternalInput").ap()
    qsa = nc.dram_tensor("qsa", (1, QS), F32R, kind="ExternalInput").ap()
    out_d = nc.dram_tensor("out", (B, QS), F32, kind="ExternalOutput").ap()

    ctx = ExitStack()
    with tile.TileContext(nc) as tc:
        cpool = ctx.enter_context(tc.tile_pool(name="consts", bufs=1))
        featp = ctx.enter_context(tc.tile_pool(name="feat", bufs=1))
        kvp = ctx.enter_context(tc.tile_pool(name="kv", bufs=1))
        maskp = ctx.enter_context(tc.tile_pool(name="mask", bufs=1))
        ep = ctx.enter_context(tc.tile_pool(name="ep", bufs=3))
        miscp = ctx.enter_context(tc.tile_pool(name="misc", bufs=2))
        pp = ctx.enter_context(tc.tile_pool(name="pp", bufs=4))
        onp = ctx.enter_context(tc.tile_pool(name="on", bufs=1))
        mlp = ctx.enter_context(tc.tile_pool(name="mlt", bufs=2))
        p_big = ctx.enter_context(tc.tile_pool(name="pbig", bufs=3, space="PSUM"))
        p_av = ctx.enter_context(tc.tile_pool(name="pav", bufs=3, space="PSUM"))
        p_row = ctx.enter_context(tc.tile_pool(name="prow", bufs=2, space="PSUM"))

        qg = cpool.tile([4, QS], F32R, tag="qg", name="qg")
        nc.sync.dma_start(qg[:], qgrid[:])
        wt0 = cpool.tile([128, W_TOTALS[0]], F32R, tag="wt0", name="wt0")
        nc.sync.dma_start(wt0[:], wp[0][:])
        qs = cpool.tile([1, QS], F32R, tag="qs", name="qs")
        nc.sync.dma_start(qs[:], qsa[:])
        tokt = [cpool.tile([128, 2 * LT], F32R, tag=f"tokt{b}", name=f"tokt{b}")
                for b in range(B)]
        for b in range(B):
            nc.scalar.dma_start(tokt[b][:], tokp[b][:])
        wt1 = cpool.tile([128, W_TOTALS[1]], F32R, tag="wt1", name="wt1")
        nc.scalar.dma_start(wt1[:], wp[1][:])
        wt2 = cpool.tile([128, W_TOTALS[2]], F32R, tag="wt2", name="wt2")
        nc.scalar.dma_start(wt2[:], wp[2][:])
        wts = [wt0, wt1, wt2]

        def wsl(name, p0=0, np_=128, sub=None):
            g, c0, ncols = W_LAYOUT[name]
            t_ = wts[g]
            if sub is not None:
                c0, ncols = c0 + sub[0], sub[1]
            return t_[p0 : p0 + np_, c0 : c0 + ncols]

        def rf(ap):
            return ap.bitcast(F32)

        warm = cpool.tile([128, QS], F32, tag="warm", name="warm")
        nc.gpsimd.memset(warm[:], 0.0)
        for wi in range(6):
            pw = p_big.tile([128, QS], F32, tag="st", name=f"warm{wi}")
            nc.tensor.matmul(pw[:], warm[:, 0:128], warm[:], start=True, stop=True)
        iotaP = cpool.tile([128, QS], F32, tag="iotap", name="iotap")
        nc.gpsimd.iota(iotaP[:], pattern=[[0, QS]], base=0, channel_multiplier=1,
                       allow_small_or_imprecise_dtypes=True)

        t_V = [[kvp.tile([128, 130], F32R, tag=f"V{b}{c}", name=f"V{b}{c}")
                for c in range(nch)] for b in range(B)]
        for b in range(B):
            for c in range(nch):
                nc.scalar.copy(t_V[b][c][:, 64:65], rf(wsl("onescol")))
                nc.scalar.copy(t_V[b][c][:, 129:130], rf(wsl("onescol")))

        def gamma_T(w2name, tag):
            pa = p_big.tile([128, QS], F32, tag="st", name=f"pa_{tag}")
            nc.tensor.matmul(pa[:64, :], rf(wsl(w2name, 0, 4)), rf(qg[:]),
                             start=True, stop=True)
            u0 = featp.tile([64, QS], F32, tag=f"{tag}_u0", name=f"{tag}_u0")
            nc.vector.tensor_scalar(u0[:], pa[:64, :], rf(wsl("scb", 0, 64)),
                                    None, OP.add)
            kf = featp.tile([64, QS], F32, tag=f"{tag}_kf", name=f"{tag}_kf")
            nc.vector.tensor_scalar(kf[:], u0[:], MAGIC, MAGIC, OP.add, OP.subtract)
            f = featp.tile([64, QS], F32, tag=f"{tag}_f", name=f"{tag}_f")
            nc.vector.tensor_tensor(f[:], u0[:], kf[:], OP.subtract)
            g = featp.tile([64, QS], F32R, tag=f"{tag}_g", name=f"{tag}_g")
            nc.scalar.activation(g[:], f[:], AF.Sin, scale=TWO_PI)
            return g

        gq = gamma_T("w2q", "gq")
        gb1 = gamma_T("w2b", "gb1")

        t_KT = [kvp.tile([128, LT], F32R, tag=f"KT{b}", name=f"KT{b}")
                for b in range(B)]
        for b in range(B):
            pk = p_big.tile([128, LT], F32, tag="st", name=f"pk{b}")
            for k in range(2):
                nc.tensor.matmul(pk[:], wsl(f"kvW{k}", 0, 128, (0, 128)),
                                 tokt[b][:, k * LT : (k + 1) * LT],
                                 start=(k == 0), stop=(k == 1))
            nc.scalar.copy(t_KT[b][:], pk[:])
            for c in range(nch):
                pvt = p_big.tile([128, QS], F32, tag="st", name=f"pv{b}{c}")
                pv = pvt[:, 0:128]
                for k in range(2):
                    nc.tensor.matmul(
                        pv, tokt[b][:, k * LT + c * 128 : k * LT + c * 128 + 128],
                        wsl(f"kvW{k}", 0, 128, (128, 128)),
                        start=(k == 0), stop=(k == 1))
                nc.vector.tensor_copy(t_V[b][c][:, 0:64], pvt[:, 0:64])
                nc.vector.tensor_copy(t_V[b][c][:, 65:129], pvt[:, 64:128])

        h_lT = [[featp.tile([128, QS], F32, tag=f"hl{l}{i}", name=f"hl{l}{i}")
                 for i in range(2)] for l in range(2)]
        x_qT = [featp.tile([128, QS], F32R, tag=f"xq{i}", name=f"xq{i}")
                for i in range(2)]
        for i in range(2):
            px = p_big.tile([128, QS], F32, tag="st", name=f"px{i}")
            nc.tensor.matmul(px[:], wsl("queryW", 0, 64, (i * 128, 128)),
                             gq[:], start=True, stop=True)
            nc.scalar.activation(x_qT[i][:], px[:], AF.Relu,
                                 bias=rf(wsl("qb", 0, 128, (i, 1))))
        for i in range(2):
            ph = p_big.tile([128, QS], F32, tag="st", name=f"ph0{i}")
            nc.tensor.matmul(ph[:], wsl("bandW", 0, 64, (0 * 256 + i * 128, 128)),
                             gq[:], start=True, stop=True)
            nc.scalar.activation(h_lT[0][i][:], ph[:], AF.Relu,
                                 bias=rf(wsl("bandb", 0, 128, (0 * 2 + i, 1))))
        qT = featp.tile([INNER, QS], F32R, tag="qT", name="qT")
        pq = p_big.tile([128, QS], F32, tag="st", name="pq")
        for k in range(2):
            nc.tensor.matmul(pq[:], wsl(f"qW{k}"), x_qT[k][:],
                             start=(k == 0), stop=(k == 1))
        nc.scalar.activation(qT[:], pq[:], AF.Copy, scale=SCALE)
        for i in range(2):
            ph = p_big.tile([128, QS], F32, tag="st", name=f"ph1{i}")
            nc.tensor.matmul(ph[:], wsl("bandW", 0, 64, (1 * 256 + i * 128, 128)),
                             gb1[:], start=True, stop=True)
            nc.scalar.activation(h_lT[1][i][:], ph[:], AF.Relu,
                                 bias=rf(wsl("bandb", 0, 128, (1 * 2 + i, 1))))
        psA = p_big.tile([128, QS], F32, tag="st", name="psA")
        nc.tensor.matmul(psA[:], wsl("ones", 0, 1), qs[:], start=True, stop=True)
        D0 = maskp.tile([128, QS], F32, tag="D0", name="D0")
        nc.vector.tensor_tensor(D0[:], iotaP[:], psA[:], OP.subtract)
        t_lni = []
        for c in range(nch):
            ac = miscp.tile([128, QS], F32, tag="ac", name="ac")
            nc.scalar.activation(ac[:], D0[:], AF.Abs,
                                 bias=rf(wsl("mbias", 0, 128, (c, 1))))
            lni = maskp.tile([128, QS], F32, tag=f"lni{c}", name=f"lni{c}")
            nc.vector.tensor_scalar(lni[:], ac[:], 63.9, NEG_BIG, OP.is_gt, OP.mult)
            t_lni.append(lni)

        oN = {}

        def emit_norm(b, h, pot, inv):
            pbc = p_big.tile([64, QS], F32, tag="st", name=f"pbc{b}{h}")
            nc.tensor.matmul(pbc[:], wsl("ones", 0, 1, (0, 64)), inv[:],
                             start=True, stop=True)
            bcs = miscp.tile([64, QS], F32, tag="bcs", name="bcs")
            nc.scalar.copy(bcs[:], pbc[:])
            onh = onp.tile([64, QS], F32R, tag=f"on{b}{h}", name=f"on{b}{h}")
            nc.vector.tensor_tensor(onh[:], pot[0:64, :], bcs[:], OP.mult)
            oN[(b, h)] = onh

        pend = None
        for b in range(B):
            for h in range(HEADS):
                pot = p_av.tile([65, QS], F32, tag="ot", name=f"ot{b}{h}")
                for c in range(nch):
                    st = p_big.tile([128, QS], F32, tag="st", name=f"st{b}{h}{c}")
                    nc.tensor.matmul(
                        st[:],
                        t_KT[b][h * 64 : (h + 1) * 64, c * 128 : (c + 1) * 128],
                        qT[h * 64 : (h + 1) * 64, :],
                        start=True, stop=True)
                    sm = ep.tile([128, QS], F32, tag="sm", name="sm")
                    nc.any.tensor_tensor(sm[:], st[:], t_lni[c][:], OP.add)
                    P = pp.tile([128, QS], F32R, tag="P", name="P")
                    nc.scalar.activation(P[:], sm[:], AF.Exp)
                    nc.tensor.matmul(
                        pot[:], t_V[b][c][:, h * 65 : (h + 1) * 65], P[:],
                        start=(c == 0), stop=(c == nch - 1))
                dr = miscp.tile([1, QS], F32, tag="dr", name="dr")
                nc.any.tensor_copy(dr[:], pot[64:65, :])
                nc.scalar.activation(dr[:], dr[:], AF.Ln)
                inv = miscp.tile([1, QS], F32R, tag="inv", name="inv")
                nc.scalar.activation(inv[:], dr[:], AF.Exp, scale=-1.0)
                if pend is not None:
                    emit_norm(*pend)
                pend = (b, h, pot, inv)

        modT = {}
        for b in range(B):
            for mc in range(2):
                pm = p_big.tile([128, QS], F32, tag="st", name=f"pm{b}{mc}")
                for h in range(2):
                    nc.tensor.matmul(
                        pm[:], wsl("outWh", 0, 64, (h * 256 + mc * 128, 128)),
                        oN[(b, h)][:], start=(h == 0), stop=(h == 1))
                mt = mlp.tile([128, QS], F32R, tag=f"modT{b}{mc}", name=f"modT{b}{mc}")
                nc.scalar.activation(mt[:], pm[:], AF.Identity,
                                     bias=rf(wsl("outb", 0, 128, (mc, 1))))
                modT[(b, mc)] = mt
            if b == 0:
                emit_norm(*pend)
        mlsd = {}
        for b in range(B):
            for l in range(2):
                for mc in range(2):
                    pm = p_big.tile([128, QS], F32, tag="st", name=f"pml{b}{l}{mc}")
                    for k in range(2):
                        nc.tensor.matmul(
                            pm[:], wsl("modW", 0, 128,
                                       ((l * 2 + k) * 256 + mc * 128, 128)),
                            modT[(b, k)][:], start=(k == 0), stop=(k == 1))
                    tadd = miscp.tile([128, QS], F32, tag="tadd", name="tadd")
                    nc.any.tensor_tensor(tadd[:], pm[:], h_lT[l][mc][:], OP.add)
                    ml = mlp.tile([128, QS], F32R, tag=f"ml{b}{l}{mc}",
                                  name=f"ml{b}{l}{mc}")
                    nc.scalar.activation(ml[:], tadd[:], AF.Relu,
                                         bias=rf(wsl("modb", 0, 128, (l * 2 + mc, 1))))
                    mlsd[(b, l, mc)] = ml
        sum01 = {}
        for b in range(B):
            for mc in range(2):
                s01 = miscp.tile([128, QS], F32R, tag=f"s01{b}{mc}", name=f"s01{b}{mc}")
                nc.any.tensor_tensor(s01[:], rf(mlsd[(b, 0, mc)][:]),
                                     rf(mlsd[(b, 1, mc)][:]), OP.add)
                sum01[(b, mc)] = s01
        hv1 = {}
        for b in range(B):
            for mc in range(2):
                pm = p_big.tile([128, QS], F32, tag="st", name=f"phv{b}{mc}")
                for k in range(2):
                    nc.tensor.matmul(
                        pm[:], wsl("hvW", 0, 128, (k * 256 + mc * 128, 128)),
                        sum01[(b, k)][:], start=(k == 0), stop=(k == 1))
                hv = mlp.tile([128, QS], F32R, tag=f"hv{b}{mc}", name=f"hv{b}{mc}")
                nc.scalar.activation(hv[:], pm[:], AF.Relu,
                                     bias=rf(wsl("hvb", 0, 128, (mc, 1))))
                hv1[(b, mc)] = hv
        for b in range(B):
            por = p_row.tile([1, QS], F32, tag="por", name=f"por{b}")
            steps = [(wsl("olW", 0, 128, (2 * k, 1)), mlsd[(b, 0, k)][:]) for k in range(2)] + \
                    [(wsl("olW", 0, 128, (2 * k + 1, 1)), hv1[(b, k)][:]) for k in range(2)] + \
                    [(wsl("olb", 0, 1), wsl("onesq", 0, 1))]
            for si, (lw, rv) in enumerate(steps):
                nc.tensor.matmul(por[:], lw, rv, start=(si == 0),
                                 stop=(si == len(steps) - 1))
            orow = mlp.tile([1, QS], F32, tag=f"orow{b}", name=f"orow{b}")
            nc.vector.tensor_copy(orow[:], por[:])
            nc.sync.dma_start(out_d[b : b + 1, :], orow[:])
        ctx.close()

    _split_multi_waits_inline(nc)
    return nc


def _split_multi_waits_inline(nc):
    """Split multi-semaphore waits into NOP chains (one wait each)."""
    for fn in nc.m.functions:
        for blk in fn.blocks:
            new_insts = []
            for inst in blk.instructions:
                si = getattr(inst, "sync_info", None)
                if si is not None and len(si.on_wait) > 1:
                    waits = list(si.on_wait)
                    for j, w in enumerate(waits[:-1]):
                        new_insts.append(mybir.InstNoOp(
                            name=f"{inst.name}-ws{j}",
                            engine=inst.engine,
                            sync_info=mybir.SyncInfo(on_wait=[w], on_update=[]),
                            bass_nofuse=True,
                        ))
                    si.on_wait = waits[-1:]
                new_insts.append(inst)
            blk.instructions = new_insts


_CACHED = {}
LAST_RESULTS = None


def _run_v4(inputs, x0, s_all, order):
    """Half-based program; returns None if the input needs the fallback."""
    cbases = []          # (core, half) -> token base
    for c in range(N_CORES):
        for x in range(2):
            qidx = order[c * QS + x * HQ : c * QS + (x + 1) * HQ]
            smin, smax = int(s_all[qidx].min()), int(s_all[qidx].max())
            cb = max(0, min(smin, L - 2 * TOPK))
            if smax - cb > TOPK:          # window must fit 2 chunks
                return None
            cbases.append(cb)

    if "v4" not in _CACHED:
        _CACHED["v4"] = build_program4()
    nc = _CACHED["v4"]

    wpacks = _pack_weights4(inputs)
    tokens = np.asarray(inputs["tokens"], np.float32)
    tt = np.ascontiguousarray(tokens.transpose(0, 2, 1)).astype(np.float16)
    LTH = 2 * TOPK

    in_maps = []
    for c in range(N_CORES):
        qidx = order[c * QS : (c + 1) * QS]
        cbA, cbB = cbases[2 * c], cbases[2 * c + 1]
        srel = np.concatenate([
            (s_all[qidx[:HQ]] - cbA), (s_all[qidx[HQ:]] - cbB)]).astype(np.float32)
        g5 = np.concatenate([x0[qidx].T, np.ones((1, QS), np.float32)], axis=0)
        m = {"wf": wpacks[0], "wb1": wpacks[1], "wb2": wpacks[2],
             "qgrid": np.ascontiguousarray(g5),
             "qsa": np.ascontiguousarray(srel[None, :])}
        for b in range(B):
            m[f"tokpack{b}"] = np.ascontiguousarray(np.concatenate(
                [tt[b, 0:128, cbA : cbA + LTH],
                 tt[b, 128:256, cbA : cbA + LTH],
                 tt[b, 0:128, cbB : cbB + LTH],
                 tt[b, 128:256, cbB : cbB + LTH]], axis=1))
        in_maps.append(m)

    trace = bool(os.environ.get("KERNEL_TRACE"))
    return run_bass_kernel_spmd(nc, in_maps, core_ids=list(range(N_CORES)),
                                trace=trace)


def _run_v3(inputs, x0, s_all, order):
    cbases, needs = [], []
    for c in range(N_CORES):
        qidx = order[c * QS : (c + 1) * QS]
        smin, smax = int(s_all[qidx].min()), int(s_all[qidx].max())
        needs.append(smax + TOPK - smin)
        cbases.append(smin)
    nch = max(3, int(math.ceil(max(needs) / 128.0)))
    LT = nch * 128
    cbases = [max(0, min(cb, L - LT)) for cb in cbases]

    key = ("v3", nch)
    if key not in _CACHED:
        _CACHED[key] = build_program3(nch)
    nc = _CACHED[key]

    wpacks = _pack_weights3(inputs)
    tokens = np.asarray(inputs["tokens"], np.float32)
    tt = tokens.transpose(0, 2, 1)

    in_maps = []
    for c in range(N_CORES):
        qidx = order[c * QS : (c + 1) * QS]
        cb = cbases[c]
        m = {"wpack0": wpacks[0], "wpack1": wpacks[1], "wpack2": wpacks[2],
             "qgrid": np.ascontiguousarray(x0[qidx].T),
             "qsa": np.ascontiguousarray(
                 (s_all[qidx] - cb).astype(np.float32)[None, :])}
        for b in range(B):
            m[f"tokpack{b}"] = np.ascontiguousarray(
                np.concatenate([tt[b, 0:128, cb : cb + LT],
                                tt[b, 128:256, cb : cb + LT]], axis=1))
        in_maps.append(m)

    trace = bool(os.environ.get("KERNEL_TRACE"))
    return run_bass_kernel_spmd(nc, in_maps, core_ids=list(range(N_CORES)),
                                trace=trace)


def kernel(**inputs):
    x = np.asarray(inputs["x"], np.float32)
    assert int(inputs["gD"]) == 8 and int(inputs["gH"]) == 8
    assert int(inputs["gW"]) == 8 and int(inputs["gT"]) == 8

    x0 = np.ascontiguousarray(x[0])  # (Q, 4) -- reference uses x[0] for all batches
    s_all = _window_starts(x0)
    order = np.argsort(s_all, kind="stable")

    global LAST_RESULTS
    res = _run_v4(inputs, x0, s_all, order)
    if res is None:
        res = _run_v3(inputs, x0, s_all, order)
    LAST_RESULTS = res
    out = np.empty((B, Q), np.float32)
    for c in range(N_CORES):
        out[:, order[c * QS : (c + 1) * QS]] = res.results[c]["out"]
    return out.reshape(B, Q, 1)
